# revision 23
# baseline (speedup 1.0000x reference)
"""Trainium2 Bass kernel for the DTGL GCN+windowed-LSTM module (bf16 rewrite).

Computation (see reference):
  h = relu(adj @ (x @ Wg0 + bg0));  h = relu(adj @ (h @ Wg1 + bg1))
  for p in 1..4: run LSTM_p over disjoint length-p windows of h (zero init
  state), writing the last hidden state back at each window end (in place).

Sharding: pure data-parallel over batch B=64 across 8 cores (8 batches per
core); adj and all weights replicated. No collectives.

Perf design vs the fp32r baseline:
  - All matmul operands bf16 (PSUM accumulation stays f32): halves the
    moving-stream bytes, halves LDWEIGHTS time, halves SBUF/DMA footprint,
    and lowers PE power so the HAM clock-gate stays at full rate.
  - adjT is SBUF-resident (loaded once, bf16, 64KB/partition).
  - Software-pipelined phases: 1B(j-1) matmuls fill the PE pipe while
    1A(j)'s PSUM->SBUF copies drain (same for 2A/2B), so the PE never
    waits on a copy.
  - LSTM runs batches round-robin per timestep so PE matmuls of batch j+1
    overlap ACT/DVE/Pool cell math of batch j. Gate PSUM uses all 8 banks.
  - Output transpose via identity matmul (stationary = h2T block, moving =
    identity) producing f32 PSUM directly.
"""

import numpy as np

B, T, D, H = 64, 2048, 256, 256
MAX_SKIP = 4
NCORES = 8
BL = B // NCORES          # batches per core
G = 4                     # batches per group
NGRP = BL // G
TK = T // 128             # 16 t-chunks
JB0 = {"i": 0, "f": 2, "g": 4, "o": 6}
XS = 16.0                 # fp8 quantization scales: x, adjT, h1
AS = 8192.0
HS = 64.0

_COMPILED = None


def _build_program():
    import concourse.mybir as mybir
    import concourse.tile as tile
    from concourse import bacc

    f32 = mybir.dt.float32
    bf16 = mybir.dt.bfloat16
    f8 = mybir.dt.float8e4

    nc = bacc.Bacc("TRN2", target_bir_lowering=False, debug=False)

    io = dict(
        x=nc.dram_tensor("x", [BL, T, D], f8, kind="ExternalInput").ap(),
        adjT=nc.dram_tensor("adjT", [T, T], f8, kind="ExternalInput").ap(),
        rs=nc.dram_tensor("rs", [1, T], bf16, kind="ExternalInput").ap(),
        wg0=nc.dram_tensor("wg0", [D, H], bf16, kind="ExternalInput").ap(),
        wg1=nc.dram_tensor("wg1", [D, H], bf16, kind="ExternalInput").ap(),
        bg0=nc.dram_tensor("bg0", [1, H], bf16, kind="ExternalInput").ap(),
        bg1=nc.dram_tensor("bg1", [1, H], bf16, kind="ExternalInput").ap(),
        wihT=nc.dram_tensor("wihT", [MAX_SKIP, H, 4 * H], bf16, kind="ExternalInput").ap(),
        whhT=nc.dram_tensor("whhT", [MAX_SKIP, H, 4 * H], bf16, kind="ExternalInput").ap(),
        biasT=nc.dram_tensor("biasT", [128, MAX_SKIP * 8], f32, kind="ExternalInput").ap(),
        out=nc.dram_tensor("out", [BL, T, D], f32, kind="ExternalOutput").ap(),
    )

    with tile.TileContext(nc) as tc:
        _emit(nc, tc, mybir, io)

    nc.compile()
    return nc


def _emit(nc, tc, mybir, io):
    from contextlib import ExitStack
    from concourse.masks import make_identity

    f32 = mybir.dt.float32
    bf16 = mybir.dt.bfloat16
    f8 = mybir.dt.float8e4
    AF = mybir.ActivationFunctionType

    with ExitStack() as root:
        cp = root.enter_context(tc.tile_pool(name="const", bufs=1))
        # adjT resident: 8 pair-tiles [128, 2*2048] fp8 (ksub-plane-major) for
        # DoubleRow matmuls: plane ks covers rows (2m+ks)*128..(2m+ks+1)*128.
        adjt = []
        for m in range(TK // 2):
            a = cp.tile([128, 2 * T], f8, name=f"adjt_{m}")
            for ks in range(2):
                nc.sync.dma_start(
                    out=a[:, ks * T:(ks + 1) * T],
                    in_=io["adjT"][(2 * m + ks) * 128:(2 * m + ks + 1) * 128, :])
            adjt.append(a)
        wg0_sb = cp.tile([128, 2 * H], bf16, name="wg0_sb")
        wg1_sb = cp.tile([128, 2 * H], bf16, name="wg1_sb")
        for dk in range(2):
            nc.sync.dma_start(out=wg0_sb[:, dk * H:(dk + 1) * H],
                              in_=io["wg0"][dk * 128:(dk + 1) * 128, :])
            nc.sync.dma_start(out=wg1_sb[:, dk * H:(dk + 1) * H],
                              in_=io["wg1"][dk * 128:(dk + 1) * 128, :])
        bg0_sb = cp.tile([1, H], bf16, name="bg0_sb")
        bg1_sb = cp.tile([1, H], bf16, name="bg1_sb")
        rs_sb = cp.tile([1, T], bf16, name="rs_sb")
        biasT_sb = cp.tile([128, MAX_SKIP * 8], f32, name="biasT_sb")
        nc.sync.dma_start(out=bg0_sb[:], in_=io["bg0"][:])
        nc.sync.dma_start(out=bg1_sb[:], in_=io["bg1"][:])
        nc.sync.dma_start(out=rs_sb[:], in_=io["rs"][:])
        nc.sync.dma_start(out=biasT_sb[:], in_=io["biasT"][:])
        id32 = cp.tile([128, 128], f32, name="id32")
        ident = cp.tile([128, 128], bf16, name="ident")
        make_identity(nc, id32[:])
        nc.vector.tensor_copy(ident[:], id32[:])

        h2t_pool = root.enter_context(tc.tile_pool(name="h2tp", bufs=1))
        lw_pool = root.enter_context(tc.tile_pool(name="lw", bufs=2))

        for grp in range(NGRP):
            _group(nc, tc, io, f32, bf16, AF, grp, adjt, wg0_sb, wg1_sb,
                   bg0_sb, bg1_sb, rs_sb, biasT_sb, ident, h2t_pool, lw_pool)


def _group(nc, tc, io, f32, bf16, AF, grp, adjt, wg0_sb, wg1_sb, bg0_sb,
           bg1_sb, rs_sb, biasT_sb, ident, h2t_pool, lw_pool):
    from contextlib import ExitStack
    import concourse.mybir as mybir

    f8 = mybir.dt.float8e4
    DR = mybir.MatmulPerfMode.DoubleRow
    bs = grp * G
    # h2T slabs: feature-major [h(part within hk), hk*T + t], bf16.
    h2t = [h2t_pool.tile([128, 2 * T], bf16, name=f"h2t_{j}", tag=f"h2t_{j}")
           for j in range(G)]

    with ExitStack() as gcn:
        h1_pool = gcn.enter_context(tc.tile_pool(name="h1p", bufs=1))
        # h1 slabs: row-major [u(part within ub), ub*H + h], fp8 scaled by HS.
        h1 = [h1_pool.tile([128, TK * H], f8, name=f"h1_{j}", tag=f"h1_{j}")
              for j in range(G)]
        x_pool = gcn.enter_context(tc.tile_pool(name="xp", bufs=2))
        z1_pool = gcn.enter_context(tc.tile_pool(name="z1p", bufs=2))

        # ---------------- Phase 1: layer 1 (1A + 1B pipelined) ----------------
        with ExitStack() as ph:
            zps = ph.enter_context(tc.tile_pool(name="zps", bufs=1, space="PSUM"))
            hps = ph.enter_context(tc.tile_pool(name="hps", bufs=4, space="PSUM"))

            xs = []
            for j in range(G):
                xt = x_pool.tile([128, TK * D], f8, name=f"x_{j}", tag="xs")
                nc.sync.dma_start(
                    out=xt[:].rearrange("p (k d) -> p k d", d=D),
                    in_=io["x"][bs + j].rearrange("(k p) d -> p k d", p=128))
                xs.append(xt)

            z1t = {}   # (j, uh, dk) -> sbuf tile [128, 1024] bf16
            pend = []  # queue of emitted-1A halves awaiting 1B: (j, uh)

            def emit_1a(j, uh):
                zp = {(dk, q): zps.tile([128, 512], f32, name="zp", tag=f"zp{dk}{q}")
                      for dk in range(2) for q in range(2)}
                xv = xs[j][:].rearrange("p (k d) -> p k d", d=D)
                for m in range(TK // 2):
                    av = adjt[m][:].rearrange("p (k u) -> p k u", k=2)
                    for dk in range(2):
                        lhs = xv[:, 2 * m:2 * m + 2, dk * 128:(dk + 1) * 128]
                        for q in range(2):
                            us = uh * 1024 + q * 512
                            nc.tensor.matmul(
                                zp[(dk, q)][:], lhs, av[:, :, us:us + 512],
                                start=(m == 0), stop=(m == TK // 2 - 1),
                                perf_mode=DR)
                for dk in range(2):
                    zt = z1_pool.tile([128, 1024], bf16, name="z1t", tag=f"z1t{dk}")
                    z1t[(j, uh, dk)] = zt
                    nc.vector.tensor_scalar_mul(zt[:, 0:512], zp[(dk, 0)][:], 1.0 / (XS * AS))
                    nc.vector.tensor_scalar_mul(zt[:, 512:1024], zp[(dk, 1)][:], 1.0 / (XS * AS))

            def emit_1b(j, uh):
                for ub_l in range(8):
                    ub = uh * 8 + ub_l
                    hp = hps.tile([128, H], f32, name="hp", tag="hp")
                    for dk in range(2):
                        nc.tensor.matmul(
                            hp[:], z1t[(j, uh, dk)][:, ub_l * 128:(ub_l + 1) * 128],
                            wg0_sb[:, dk * H:(dk + 1) * H],
                            start=(dk == 0), stop=False)
                    nc.tensor.matmul(hp[:], rs_sb[0:1, ub * 128:(ub + 1) * 128],
                                     bg0_sb[0:1, :], start=False, stop=True)
                    # h1 = relu(HS * z) in fp8 (scale folded through relu)
                    nc.scalar.activation(h1[j][:, ub * H:(ub + 1) * H], hp[:],
                                         AF.Relu, scale=HS)

            for j in range(G):
                for uh in range(2):
                    emit_1a(j, uh)
                    pend.append((j, uh))
                    if len(pend) > 1:
                        emit_1b(*pend.pop(0))
            while pend:
                emit_1b(*pend.pop(0))

        # ---------------- Phase 2: layer 2 (2A + 2B pipelined) ----------------
        with ExitStack() as ph:
            zps = ph.enter_context(tc.tile_pool(name="zps2", bufs=1, space="PSUM"))
            hps = ph.enter_context(tc.tile_pool(name="hps2", bufs=1, space="PSUM"))
            z2_pool = ph.enter_context(tc.tile_pool(name="z2p", bufs=2))

            z2t = {}
            pend = []

            def emit_2a(j, uh):
                zp = {(hk, q): zps.tile([128, 512], f32, name="zp2", tag=f"zp2{hk}{q}")
                      for hk in range(2) for q in range(2)}
                hv = h1[j][:].rearrange("p (u h) -> p u h", h=H)
                for m in range(TK // 2):
                    av = adjt[m][:].rearrange("p (k u) -> p k u", k=2)
                    for hk in range(2):
                        lhs = hv[:, 2 * m:2 * m + 2, hk * 128:(hk + 1) * 128]
                        for q in range(2):
                            us = uh * 1024 + q * 512
                            nc.tensor.matmul(
                                zp[(hk, q)][:], lhs, av[:, :, us:us + 512],
                                start=(m == 0), stop=(m == TK // 2 - 1),
                                perf_mode=DR)
                for hk in range(2):
                    zt = z2_pool.tile([128, 1024], bf16, name="z2t", tag=f"z2t{hk}")
                    z2t[(j, uh, hk)] = zt
                    nc.vector.tensor_scalar_mul(zt[:, 0:512], zp[(hk, 0)][:], 1.0 / (HS * AS))
                    nc.vector.tensor_scalar_mul(zt[:, 512:1024], zp[(hk, 1)][:], 1.0 / (HS * AS))

            def emit_2b(j, uh):
                for ho in range(2):
                    for q in range(2):
                        hp = hps.tile([128, 512], f32, name="hp2", tag=f"hp2{ho}{q}")
                        for hk in range(2):
                            nc.tensor.matmul(
                                hp[:], wg1_sb[:, hk * H + ho * 128: hk * H + (ho + 1) * 128],
                                z2t[(j, uh, hk)][:, q * 512:(q + 1) * 512],
                                start=(hk == 0), stop=False)
                        us = uh * 1024 + q * 512
                        nc.tensor.matmul(hp[:], bg1_sb[0:1, ho * 128:(ho + 1) * 128],
                                         rs_sb[0:1, us:us + 512], start=False, stop=True)
                        nc.vector.tensor_relu(h2t[j][:, ho * T + us: ho * T + us + 512],
                                              hp[:])

            for j in range(G):
                for uh in range(2):
                    emit_2a(j, uh)
                    pend.append((j, uh))
                    if len(pend) > 1:
                        emit_2b(*pend.pop(0))
            while pend:
                emit_2b(*pend.pop(0))

    # ---------------- Phases 3-4: the four LSTM passes ----------------
    # Two cohorts (j0,j1 | j2,j3) run with a one-pass skew so ACT-heavy t=0
    # units (esp. all of p=1) interleave with PE-heavy t>0 units.
    with ExitStack() as ph:
        gps = ph.enter_context(tc.tile_pool(name="gps", bufs=1, space="PSUM"))
        gsb = ph.enter_context(tc.tile_pool(name="gsb", bufs=3))
        st_pool = ph.enter_context(tc.tile_pool(name="st", bufs=1))
        h_pool = ph.enter_context(tc.tile_pool(name="hs", bufs=2))
        gx_pool = ph.enter_context(tc.tile_pool(name="gx", bufs=3))

        c_st = [st_pool.tile([128, 1024], bf16, name=f"c_{j}", tag=f"c{j}")
                for j in range(G)]
        h_t = [None] * G
        lw = {}

        def get_weights(p):
            if p in lw:
                return lw[p]
            wih = lw_pool.tile([128, 2 * 4 * H], bf16, name=f"wih{grp}{p}", tag="wih")
            for hk in range(2):
                nc.sync.dma_start(out=wih[:, hk * 4 * H:(hk + 1) * 4 * H],
                                  in_=io["wihT"][p - 1, hk * 128:(hk + 1) * 128, :])
            whh = None
            if p > 1:
                whh = lw_pool.tile([128, 2 * 4 * H], bf16, name=f"whh{grp}{p}", tag="whh")
                for hk in range(2):
                    nc.sync.dma_start(out=whh[:, hk * 4 * H:(hk + 1) * 4 * H],
                                      in_=io["whhT"][p - 1, hk * 128:(hk + 1) * 128, :])
            lw[p] = (wih, whh)
            return lw[p]

        def emit_step(p, ws, t, j):
            nw = T // p
            ncw = min(512, nw - ws)
            spans = ([slice(0, 1024)] if ncw == 512
                     else [slice(0, ncw), slice(512, 512 + ncw)])
            wih, whh = get_weights(p)
            view = [h2t[j][:, hk * T: hk * T + nw * p].rearrange(
                "a (w q) -> a w q", q=p) for hk in range(2)]
            if True:
                    if True:
                        if p > 1:
                            xc = gx_pool.tile([128, 1024], bf16, name="xc", tag="xc")
                            nc.gpsimd.tensor_copy(xc[:, 0:ncw],
                                                  view[0][:, ws:ws + ncw, t:t + 1])
                            nc.vector.tensor_copy(xc[:, 512:512 + ncw],
                                                  view[1][:, ws:ws + ncw, t:t + 1])
                        gates = "igo" if t == 0 else "ifgo"
                        gp = {}
                        # input-weight matmuls first (no state dependency)
                        for gn in gates:
                            psum = gps.tile([128, 1024], f32, name=f"ps_{gn}", tag=f"ps_{gn}")
                            gp[gn] = psum
                            for half in range(2):
                                jb = JB0[gn] + half
                                o = psum[:, half * 512: half * 512 + ncw]
                                for hk in range(2):
                                    rhs = (view[hk][:, ws:ws + ncw, 0:1] if p == 1
                                           else xc[:, hk * 512: hk * 512 + ncw])
                                    nc.tensor.matmul(
                                        o,
                                        wih[:, hk * 4 * H + jb * 128: hk * 4 * H + (jb + 1) * 128],
                                        rhs,
                                        start=(hk == 0),
                                        stop=(t == 0 and hk == 1))
                        if t > 0:
                            for gn in gates:
                                for half in range(2):
                                    jb = JB0[gn] + half
                                    o = gp[gn][:, half * 512: half * 512 + ncw]
                                    for hk in range(2):
                                        nc.tensor.matmul(
                                            o,
                                            whh[:, hk * 4 * H + jb * 128: hk * 4 * H + (jb + 1) * 128],
                                            h_t[j][:, hk * 512: hk * 512 + ncw],
                                            start=False, stop=(hk == 1))
                        act = {}
                        for gn in gates:
                            fn = AF.Tanh if gn == "g" else AF.Sigmoid
                            a = gsb.tile([128, 1024], bf16, name=f"a_{gn}", tag=f"a_{gn}")
                            act[gn] = a
                            for half in range(2):
                                col = (p - 1) * 8 + JB0[gn] + half
                                nc.scalar.activation(
                                    a[:, half * 512: half * 512 + ncw],
                                    gp[gn][:, half * 512: half * 512 + ncw],
                                    fn, bias=biasT_sb[:, col:col + 1])
                        cn = c_st[j]
                        if t == 0:
                            for s in spans:
                                nc.vector.tensor_mul(cn[:, s], act["i"][:, s], act["g"][:, s])
                        else:
                            for s in spans:
                                nc.vector.tensor_mul(act["g"][:, s], act["i"][:, s], act["g"][:, s])
                            for s in spans:
                                nc.gpsimd.tensor_mul(cn[:, s], act["f"][:, s], cn[:, s])
                            for s in spans:
                                nc.vector.tensor_add(cn[:, s], cn[:, s], act["g"][:, s])
                        # tanh(c) overwrites the i tile (free after c update)
                        tc_t = act["i"]
                        for s in spans:
                            nc.scalar.activation(tc_t[:, s], cn[:, s], AF.Tanh)
                        if t == p - 1:
                            for hk in range(2):
                                nc.vector.tensor_mul(
                                    view[hk][:, ws:ws + ncw, p - 1:p],
                                    act["o"][:, hk * 512: hk * 512 + ncw],
                                    tc_t[:, hk * 512: hk * 512 + ncw])
                        else:
                            hn = h_pool.tile([128, 1024], bf16, name="hn", tag=f"h{j}")
                            for s in spans:
                                nc.vector.tensor_mul(hn[:, s], act["o"][:, s], tc_t[:, s])
                            h_t[j] = hn

        def cohort_units(cjs):
            by_pass = {}
            for p in range(1, MAX_SKIP + 1):
                units = []
                nw = T // p
                for ws in range(0, nw, 512):
                    for t in range(p):
                        units.append((p, ws, t, cjs))
                by_pass[p] = units
            return by_pass

        A = cohort_units([0, 1])
        Bc = cohort_units([2, 3])
        merged = list(A[1])
        for p in range(1, MAX_SKIP + 1):
            xs_, ys_ = Bc[p], A.get(p + 1, [])
            n = max(len(xs_), len(ys_))
            for i in range(n):
                if i < len(xs_):
                    merged.append(xs_[i])
                if i < len(ys_):
                    merged.append(ys_[i])
        for (p, ws, t, cjs) in merged:
            for j in cjs:
                emit_step(p, ws, t, j)

        # ------------- Phase 5: transpose h2T -> out (reuses gate PSUM) -------------
        osb = ph.enter_context(tc.tile_pool(name="osb", bufs=2))
        tptags = ["ps_i", "ps_f", "ps_g", "ps_o"]
        for j in range(G):
            b = bs + j
            for tg in range(4):
                tp = gps.tile([128, 1024], f32, name="tp", tag=tptags[tg])
                for q in range(4):
                    tk = tg * 4 + q
                    for hk in range(2):
                        nc.tensor.matmul(
                            tp[:, q * D + hk * 128: q * D + (hk + 1) * 128],
                            h2t[j][:, hk * T + tk * 128: hk * T + (tk + 1) * 128],
                            ident[:], start=True, stop=True)
                ot = osb.tile([128, 1024], f32, name="ot", tag="ot")
                if tg % 2 == 0:
                    nc.scalar.activation(ot[:], tp[:], AF.Copy)
                else:
                    nc.vector.tensor_copy(ot[:], tp[:])
                nc.sync.dma_start(
                    out=io["out"][b, tg * 512:(tg + 1) * 512, :].rearrange(
                        "(q p) d -> p q d", p=128),
                    in_=ot[:].rearrange("p (q d) -> p q d", d=D))


def _prep_host(inputs):
    import ml_dtypes
    bf16 = ml_dtypes.bfloat16
    f8 = ml_dtypes.float8_e4m3fn

    def q8(a, scale):
        return np.ascontiguousarray(
            np.clip(np.asarray(a, dtype=np.float32) * scale, -240, 240)).astype(f8)

    x = np.asarray(inputs["x"], dtype=np.float32)
    adj = np.asarray(inputs["adj"], dtype=np.float32)
    adjT = q8(adj.T, AS)
    rs = np.ascontiguousarray(
        adj.sum(axis=1, dtype=np.float32).reshape(1, T)).astype(bf16)
    wg0 = np.ascontiguousarray(inputs["Wg0"], dtype=np.float32).astype(bf16)
    wg1 = np.ascontiguousarray(inputs["Wg1"], dtype=np.float32).astype(bf16)
    bg0 = np.ascontiguousarray(inputs["bg0"], dtype=np.float32).reshape(1, H).astype(bf16)
    bg1 = np.ascontiguousarray(inputs["bg1"], dtype=np.float32).reshape(1, H).astype(bf16)
    wihT = np.ascontiguousarray(
        np.asarray(inputs["Wih"], dtype=np.float32).transpose(0, 2, 1)).astype(bf16)
    whhT = np.ascontiguousarray(
        np.asarray(inputs["Whh"], dtype=np.float32).transpose(0, 2, 1)).astype(bf16)
    bias = np.asarray(inputs["bih"], dtype=np.float32) + np.asarray(inputs["bhh"], dtype=np.float32)
    biasT = np.ascontiguousarray(
        bias.reshape(MAX_SKIP, 8, 128).transpose(2, 0, 1).reshape(128, MAX_SKIP * 8))
    shared = dict(adjT=adjT, rs=rs, wg0=wg0, wg1=wg1, bg0=bg0, bg1=bg1,
                  wihT=wihT, whhT=whhT, biasT=biasT)
    xb = q8(x, XS)
    in_maps = []
    for c in range(NCORES):
        m = dict(shared)
        m["x"] = np.ascontiguousarray(xb[c * BL:(c + 1) * BL])
        in_maps.append(m)
    return in_maps


def get_compiled():
    global _COMPILED
    if _COMPILED is None:
        _COMPILED = _build_program()
    return _COMPILED


def kernel(**inputs) -> np.ndarray:
    from concourse.bass_utils import run_bass_kernel_spmd

    nc = get_compiled()
    in_maps = _prep_host(inputs)
    res = run_bass_kernel_spmd(nc, in_maps, list(range(NCORES)))
    out = np.concatenate([res.results[c]["out"] for c in range(NCORES)], axis=0)
    return out.astype(np.float32)


# revision 25
# speedup vs baseline: 1.0611x; 1.0611x over previous
"""Trainium2 Bass kernel for the DTGL GCN+windowed-LSTM module (bf16 rewrite).

Computation (see reference):
  h = relu(adj @ (x @ Wg0 + bg0));  h = relu(adj @ (h @ Wg1 + bg1))
  for p in 1..4: run LSTM_p over disjoint length-p windows of h (zero init
  state), writing the last hidden state back at each window end (in place).

Sharding: pure data-parallel over batch B=64 across 8 cores (8 batches per
core); adj and all weights replicated. No collectives.

Perf design vs the fp32r baseline:
  - All matmul operands bf16 (PSUM accumulation stays f32): halves the
    moving-stream bytes, halves LDWEIGHTS time, halves SBUF/DMA footprint,
    and lowers PE power so the HAM clock-gate stays at full rate.
  - adjT is SBUF-resident (loaded once, bf16, 64KB/partition).
  - Software-pipelined phases: 1B(j-1) matmuls fill the PE pipe while
    1A(j)'s PSUM->SBUF copies drain (same for 2A/2B), so the PE never
    waits on a copy.
  - LSTM runs batches round-robin per timestep so PE matmuls of batch j+1
    overlap ACT/DVE/Pool cell math of batch j. Gate PSUM uses all 8 banks.
  - Output transpose via identity matmul (stationary = h2T block, moving =
    identity) producing f32 PSUM directly.
"""

import numpy as np

B, T, D, H = 64, 2048, 256, 256
MAX_SKIP = 4
NCORES = 8
BL = B // NCORES          # batches per core
G = 4                     # batches per group
NGRP = BL // G
TK = T // 128             # 16 t-chunks
JB0 = {"i": 0, "f": 2, "g": 4, "o": 6}
XS = 16.0                 # fp8 quantization scales: x, adjT, h1
AS = 8192.0
HS = 64.0

_COMPILED = None


def _build_program():
    import concourse.mybir as mybir
    import concourse.tile as tile
    from concourse import bacc

    f32 = mybir.dt.float32
    bf16 = mybir.dt.bfloat16
    f8 = mybir.dt.float8e4

    nc = bacc.Bacc("TRN2", target_bir_lowering=False, debug=False)

    io = dict(
        x=nc.dram_tensor("x", [BL, T, D], f8, kind="ExternalInput").ap(),
        adjT=nc.dram_tensor("adjT", [T, T], f8, kind="ExternalInput").ap(),
        rs=nc.dram_tensor("rs", [1, T], bf16, kind="ExternalInput").ap(),
        wg0=nc.dram_tensor("wg0", [D, H], bf16, kind="ExternalInput").ap(),
        wg1=nc.dram_tensor("wg1", [D, H], bf16, kind="ExternalInput").ap(),
        bg0=nc.dram_tensor("bg0", [1, H], bf16, kind="ExternalInput").ap(),
        bg1=nc.dram_tensor("bg1", [1, H], bf16, kind="ExternalInput").ap(),
        wihT=nc.dram_tensor("wihT", [MAX_SKIP, H, 4 * H], bf16, kind="ExternalInput").ap(),
        whhT=nc.dram_tensor("whhT", [MAX_SKIP, H, 4 * H], bf16, kind="ExternalInput").ap(),
        biasT=nc.dram_tensor("biasT", [128, MAX_SKIP * 8], f32, kind="ExternalInput").ap(),
        out=nc.dram_tensor("out", [BL, T, D], f32, kind="ExternalOutput").ap(),
    )

    with tile.TileContext(nc) as tc:
        _emit(nc, tc, mybir, io)

    nc.compile()
    return nc


def _emit(nc, tc, mybir, io):
    from contextlib import ExitStack
    from concourse.masks import make_identity

    f32 = mybir.dt.float32
    bf16 = mybir.dt.bfloat16
    f8 = mybir.dt.float8e4
    AF = mybir.ActivationFunctionType

    with ExitStack() as root:
        cp = root.enter_context(tc.tile_pool(name="const", bufs=1))
        # adjT resident: 8 pair-tiles [128, 2*2048] fp8 (ksub-plane-major) for
        # DoubleRow matmuls: plane ks covers rows (2m+ks)*128..(2m+ks+1)*128.
        adjt = []
        for m in range(TK // 2):
            a = cp.tile([128, 2 * T], f8, name=f"adjt_{m}")
            for ks in range(2):
                nc.sync.dma_start(
                    out=a[:, ks * T:(ks + 1) * T],
                    in_=io["adjT"][(2 * m + ks) * 128:(2 * m + ks + 1) * 128, :])
            adjt.append(a)
        wg0_sb = cp.tile([128, 2 * H], bf16, name="wg0_sb")
        wg1_sb = cp.tile([128, 2 * H], bf16, name="wg1_sb")
        for dk in range(2):
            nc.sync.dma_start(out=wg0_sb[:, dk * H:(dk + 1) * H],
                              in_=io["wg0"][dk * 128:(dk + 1) * 128, :])
            nc.sync.dma_start(out=wg1_sb[:, dk * H:(dk + 1) * H],
                              in_=io["wg1"][dk * 128:(dk + 1) * 128, :])
        bg0_sb = cp.tile([1, H], bf16, name="bg0_sb")
        bg1_sb = cp.tile([1, H], bf16, name="bg1_sb")
        rs_sb = cp.tile([1, T], bf16, name="rs_sb")
        biasT_sb = cp.tile([128, MAX_SKIP * 8], f32, name="biasT_sb")
        nc.sync.dma_start(out=bg0_sb[:], in_=io["bg0"][:])
        nc.sync.dma_start(out=bg1_sb[:], in_=io["bg1"][:])
        nc.sync.dma_start(out=rs_sb[:], in_=io["rs"][:])
        nc.sync.dma_start(out=biasT_sb[:], in_=io["biasT"][:])
        id32 = cp.tile([128, 128], f32, name="id32")
        ident = cp.tile([128, 128], bf16, name="ident")
        make_identity(nc, id32[:])
        nc.vector.tensor_copy(ident[:], id32[:])

        h2t_pool = root.enter_context(tc.tile_pool(name="h2tp", bufs=1))
        lw_pool = root.enter_context(tc.tile_pool(name="lw", bufs=2))

        for grp in range(NGRP):
            _group(nc, tc, io, f32, bf16, AF, grp, adjt, wg0_sb, wg1_sb,
                   bg0_sb, bg1_sb, rs_sb, biasT_sb, ident, h2t_pool, lw_pool)


def _group(nc, tc, io, f32, bf16, AF, grp, adjt, wg0_sb, wg1_sb, bg0_sb,
           bg1_sb, rs_sb, biasT_sb, ident, h2t_pool, lw_pool):
    from contextlib import ExitStack
    import concourse.mybir as mybir

    f8 = mybir.dt.float8e4
    DR = mybir.MatmulPerfMode.DoubleRow
    bs = grp * G
    # h2T slabs: feature-major [h(part within hk), hk*T + t], bf16.
    h2t = [h2t_pool.tile([128, 2 * T], bf16, name=f"h2t_{j}", tag=f"h2t_{j}")
           for j in range(G)]

    with ExitStack() as gcn:
        h1_pool = gcn.enter_context(tc.tile_pool(name="h1p", bufs=1))
        # h1 slabs: row-major [u(part within ub), ub*H + h], fp8 scaled by HS.
        h1 = [h1_pool.tile([128, TK * H], f8, name=f"h1_{j}", tag=f"h1_{j}")
              for j in range(G)]
        x_pool = gcn.enter_context(tc.tile_pool(name="xp", bufs=2))
        z1_pool = gcn.enter_context(tc.tile_pool(name="z1p", bufs=2))

        # ---------------- Phase 1: layer 1 (1A + 1B pipelined) ----------------
        with ExitStack() as ph:
            zps = ph.enter_context(tc.tile_pool(name="zps", bufs=1, space="PSUM"))
            hps = ph.enter_context(tc.tile_pool(name="hps", bufs=4, space="PSUM"))

            xs = []
            for j in range(G):
                xt = x_pool.tile([128, TK * D], f8, name=f"x_{j}", tag="xs")
                nc.sync.dma_start(
                    out=xt[:].rearrange("p (k d) -> p k d", d=D),
                    in_=io["x"][bs + j].rearrange("(k p) d -> p k d", p=128))
                xs.append(xt)

            z1t = {}   # (j, uh, dk) -> sbuf tile [128, 1024] bf16
            pend = []  # queue of emitted-1A halves awaiting 1B: (j, uh)

            def emit_1a(j, uh):
                zp = {(dk, q): zps.tile([128, 512], f32, name="zp", tag=f"zp{dk}{q}")
                      for dk in range(2) for q in range(2)}
                xv = xs[j][:].rearrange("p (k d) -> p k d", d=D)
                for m in range(TK // 2):
                    av = adjt[m][:].rearrange("p (k u) -> p k u", k=2)
                    for dk in range(2):
                        lhs = xv[:, 2 * m:2 * m + 2, dk * 128:(dk + 1) * 128]
                        for q in range(2):
                            us = uh * 1024 + q * 512
                            nc.tensor.matmul(
                                zp[(dk, q)][:], lhs, av[:, :, us:us + 512],
                                start=(m == 0), stop=(m == TK // 2 - 1),
                                perf_mode=DR)
                for dk in range(2):
                    zt = z1_pool.tile([128, 1024], bf16, name="z1t", tag=f"z1t{dk}")
                    z1t[(j, uh, dk)] = zt
                    nc.vector.tensor_scalar_mul(zt[:, 0:512], zp[(dk, 0)][:], 1.0 / (XS * AS))
                    nc.vector.tensor_scalar_mul(zt[:, 512:1024], zp[(dk, 1)][:], 1.0 / (XS * AS))

            def emit_1b(j, uh):
                for ub_l in range(8):
                    ub = uh * 8 + ub_l
                    hp = hps.tile([128, H], f32, name="hp", tag="hp")
                    for dk in range(2):
                        nc.tensor.matmul(
                            hp[:], z1t[(j, uh, dk)][:, ub_l * 128:(ub_l + 1) * 128],
                            wg0_sb[:, dk * H:(dk + 1) * H],
                            start=(dk == 0), stop=False)
                    nc.tensor.matmul(hp[:], rs_sb[0:1, ub * 128:(ub + 1) * 128],
                                     bg0_sb[0:1, :], start=False, stop=True)
                    # h1 = relu(HS * z) in fp8 (scale folded through relu)
                    nc.scalar.activation(h1[j][:, ub * H:(ub + 1) * H], hp[:],
                                         AF.Relu, scale=HS)

            for j in range(G):
                for uh in range(2):
                    emit_1a(j, uh)
                    pend.append((j, uh))
                    if len(pend) > 1:
                        emit_1b(*pend.pop(0))
            while pend:
                emit_1b(*pend.pop(0))

        # ---------------- Phase 2: layer 2 (2A + 2B pipelined) ----------------
        with ExitStack() as ph:
            zps = ph.enter_context(tc.tile_pool(name="zps2", bufs=1, space="PSUM"))
            hps = ph.enter_context(tc.tile_pool(name="hps2", bufs=1, space="PSUM"))
            z2_pool = ph.enter_context(tc.tile_pool(name="z2p", bufs=2))

            z2t = {}
            pend = []

            def emit_2a(j, uh):
                zp = {(hk, q): zps.tile([128, 512], f32, name="zp2", tag=f"zp2{hk}{q}")
                      for hk in range(2) for q in range(2)}
                hv = h1[j][:].rearrange("p (u h) -> p u h", h=H)
                for m in range(TK // 2):
                    av = adjt[m][:].rearrange("p (k u) -> p k u", k=2)
                    for hk in range(2):
                        lhs = hv[:, 2 * m:2 * m + 2, hk * 128:(hk + 1) * 128]
                        for q in range(2):
                            us = uh * 1024 + q * 512
                            nc.tensor.matmul(
                                zp[(hk, q)][:], lhs, av[:, :, us:us + 512],
                                start=(m == 0), stop=(m == TK // 2 - 1),
                                perf_mode=DR)
                for hk in range(2):
                    zt = z2_pool.tile([128, 1024], bf16, name="z2t", tag=f"z2t{hk}")
                    z2t[(j, uh, hk)] = zt
                    nc.vector.tensor_scalar_mul(zt[:, 0:512], zp[(hk, 0)][:], 1.0 / (HS * AS))
                    nc.vector.tensor_scalar_mul(zt[:, 512:1024], zp[(hk, 1)][:], 1.0 / (HS * AS))

            def emit_2b(j, uh):
                for ho in range(2):
                    for q in range(2):
                        hp = hps.tile([128, 512], f32, name="hp2", tag=f"hp2{ho}{q}")
                        for hk in range(2):
                            nc.tensor.matmul(
                                hp[:], wg1_sb[:, hk * H + ho * 128: hk * H + (ho + 1) * 128],
                                z2t[(j, uh, hk)][:, q * 512:(q + 1) * 512],
                                start=(hk == 0), stop=False)
                        us = uh * 1024 + q * 512
                        nc.tensor.matmul(hp[:], bg1_sb[0:1, ho * 128:(ho + 1) * 128],
                                         rs_sb[0:1, us:us + 512], start=False, stop=True)
                        nc.vector.tensor_relu(h2t[j][:, ho * T + us: ho * T + us + 512],
                                              hp[:])

            for j in range(G):
                for uh in range(2):
                    emit_2a(j, uh)
                    pend.append((j, uh))
                    if len(pend) > 1:
                        emit_2b(*pend.pop(0))
            while pend:
                emit_2b(*pend.pop(0))

    # ---------------- Phases 3-4: the four LSTM passes ----------------
    # Two cohorts (j0,j1 | j2,j3) run with a one-pass skew so ACT-heavy t=0
    # units (esp. all of p=1) interleave with PE-heavy t>0 units.
    with ExitStack() as ph:
        gps = ph.enter_context(tc.tile_pool(name="gps", bufs=1, space="PSUM"))
        gsb = ph.enter_context(tc.tile_pool(name="gsb", bufs=3))
        st_pool = ph.enter_context(tc.tile_pool(name="st", bufs=1))
        h_pool = ph.enter_context(tc.tile_pool(name="hs", bufs=2))
        gx_pool = ph.enter_context(tc.tile_pool(name="gx", bufs=3))

        c_st = [st_pool.tile([128, 1024], bf16, name=f"c_{j}", tag=f"c{j}")
                for j in range(G)]
        h_t = [None] * G
        lw = {}

        def get_weights(p):
            if p in lw:
                return lw[p]
            wih = lw_pool.tile([128, 2 * 4 * H], bf16, name=f"wih{grp}{p}", tag="wih")
            for hk in range(2):
                nc.sync.dma_start(out=wih[:, hk * 4 * H:(hk + 1) * 4 * H],
                                  in_=io["wihT"][p - 1, hk * 128:(hk + 1) * 128, :])
            whh = None
            if p > 1:
                whh = lw_pool.tile([128, 2 * 4 * H], bf16, name=f"whh{grp}{p}", tag="whh")
                for hk in range(2):
                    nc.sync.dma_start(out=whh[:, hk * 4 * H:(hk + 1) * 4 * H],
                                      in_=io["whhT"][p - 1, hk * 128:(hk + 1) * 128, :])
            lw[p] = (wih, whh)
            return lw[p]

        def emit_head(p, ws, t, j):
            nw = T // p
            ncw = min(512, nw - ws)
            spans = ([slice(0, 1024)] if ncw == 512
                     else [slice(0, ncw), slice(512, 512 + ncw)])
            wih, whh = get_weights(p)
            view = [h2t[j][:, hk * T: hk * T + nw * p].rearrange(
                "a (w q) -> a w q", q=p) for hk in range(2)]
            if True:
                    if True:
                        if p > 1:
                            xc = gx_pool.tile([128, 1024], bf16, name="xc", tag="xc")
                            nc.gpsimd.tensor_copy(xc[:, 0:ncw],
                                                  view[0][:, ws:ws + ncw, t:t + 1])
                            nc.vector.tensor_copy(xc[:, 512:512 + ncw],
                                                  view[1][:, ws:ws + ncw, t:t + 1])
                        gates = "igo" if t == 0 else "ifgo"
                        gp = {}
                        # input-weight matmuls first (no state dependency)
                        for gn in gates:
                            psum = gps.tile([128, 1024], f32, name=f"ps_{gn}", tag=f"ps_{gn}")
                            gp[gn] = psum
                            for half in range(2):
                                jb = JB0[gn] + half
                                o = psum[:, half * 512: half * 512 + ncw]
                                for hk in range(2):
                                    rhs = (view[hk][:, ws:ws + ncw, 0:1] if p == 1
                                           else xc[:, hk * 512: hk * 512 + ncw])
                                    nc.tensor.matmul(
                                        o,
                                        wih[:, hk * 4 * H + jb * 128: hk * 4 * H + (jb + 1) * 128],
                                        rhs,
                                        start=(hk == 0),
                                        stop=(t == 0 and hk == 1))
                        if t > 0:
                            for gn in gates:
                                for half in range(2):
                                    jb = JB0[gn] + half
                                    o = gp[gn][:, half * 512: half * 512 + ncw]
                                    for hk in range(2):
                                        nc.tensor.matmul(
                                            o,
                                            whh[:, hk * 4 * H + jb * 128: hk * 4 * H + (jb + 1) * 128],
                                            h_t[j][:, hk * 512: hk * 512 + ncw],
                                            start=False, stop=(hk == 1))
                        act = {}
                        for gn in gates:
                            fn = AF.Tanh if gn == "g" else AF.Sigmoid
                            a = gsb.tile([128, 1024], bf16, name=f"a_{gn}", tag=f"a_{gn}")
                            act[gn] = a
                            for half in range(2):
                                col = (p - 1) * 8 + JB0[gn] + half
                                nc.scalar.activation(
                                    a[:, half * 512: half * 512 + ncw],
                                    gp[gn][:, half * 512: half * 512 + ncw],
                                    fn, bias=biasT_sb[:, col:col + 1])
                        cn = c_st[j]
                        if t == 0:
                            for s in spans:
                                nc.vector.tensor_mul(cn[:, s], act["i"][:, s], act["g"][:, s])
                        else:
                            for s in spans:
                                nc.vector.tensor_mul(act["g"][:, s], act["i"][:, s], act["g"][:, s])
                            for s in spans:
                                nc.gpsimd.tensor_mul(cn[:, s], act["f"][:, s], cn[:, s])
                            for s in spans:
                                nc.vector.tensor_add(cn[:, s], cn[:, s], act["g"][:, s])
                        # tail (tanh + h-mul) deferred one unit to avoid ACT
                        # head-of-line blocking on the not-yet-ready c input
                        return (p, ws, t, j, ncw, spans, view, act, cn)

        def emit_tail(p, ws, t, j, ncw, spans, view, act, cn):
            # tanh(c) overwrites the i tile (free after c update)
            tc_t = act["i"]
            for s in spans:
                nc.scalar.activation(tc_t[:, s], cn[:, s], AF.Tanh)
            if t == p - 1:
                for hk in range(2):
                    nc.vector.tensor_mul(
                        view[hk][:, ws:ws + ncw, p - 1:p],
                        act["o"][:, hk * 512: hk * 512 + ncw],
                        tc_t[:, hk * 512: hk * 512 + ncw])
            else:
                hn = h_pool.tile([128, 1024], bf16, name="hn", tag=f"h{j}")
                for s in spans:
                    nc.vector.tensor_mul(hn[:, s], act["o"][:, s], tc_t[:, s])
                h_t[j] = hn

        pend_tail = []
        for p in range(1, MAX_SKIP + 1):
            nw = T // p
            for ws in range(0, nw, 512):
                for t in range(p):
                    for j in range(G):
                        tail = emit_head(p, ws, t, j)
                        pend_tail.append(tail)
                        if len(pend_tail) > 1:
                            emit_tail(*pend_tail.pop(0))
        while pend_tail:
            emit_tail(*pend_tail.pop(0))

        # ------------- Phase 5: transpose h2T -> out (reuses gate PSUM) -------------
        osb = ph.enter_context(tc.tile_pool(name="osb", bufs=2))
        tptags = ["ps_i", "ps_f", "ps_g", "ps_o"]
        for j in range(G):
            b = bs + j
            for tg in range(4):
                tp = gps.tile([128, 1024], f32, name="tp", tag=tptags[tg])
                for q in range(4):
                    tk = tg * 4 + q
                    for hk in range(2):
                        nc.tensor.matmul(
                            tp[:, q * D + hk * 128: q * D + (hk + 1) * 128],
                            h2t[j][:, hk * T + tk * 128: hk * T + (tk + 1) * 128],
                            ident[:], start=True, stop=True)
                ot = osb.tile([128, 1024], f32, name="ot", tag="ot")
                if tg % 2 == 0:
                    nc.scalar.activation(ot[:], tp[:], AF.Copy)
                else:
                    nc.vector.tensor_copy(ot[:], tp[:])
                nc.sync.dma_start(
                    out=io["out"][b, tg * 512:(tg + 1) * 512, :].rearrange(
                        "(q p) d -> p q d", p=128),
                    in_=ot[:].rearrange("p (q d) -> p q d", d=D))


def _prep_host(inputs):
    import ml_dtypes
    bf16 = ml_dtypes.bfloat16
    f8 = ml_dtypes.float8_e4m3fn

    def q8(a, scale):
        return np.ascontiguousarray(
            np.clip(np.asarray(a, dtype=np.float32) * scale, -240, 240)).astype(f8)

    x = np.asarray(inputs["x"], dtype=np.float32)
    adj = np.asarray(inputs["adj"], dtype=np.float32)
    adjT = q8(adj.T, AS)
    rs = np.ascontiguousarray(
        adj.sum(axis=1, dtype=np.float32).reshape(1, T)).astype(bf16)
    wg0 = np.ascontiguousarray(inputs["Wg0"], dtype=np.float32).astype(bf16)
    wg1 = np.ascontiguousarray(inputs["Wg1"], dtype=np.float32).astype(bf16)
    bg0 = np.ascontiguousarray(inputs["bg0"], dtype=np.float32).reshape(1, H).astype(bf16)
    bg1 = np.ascontiguousarray(inputs["bg1"], dtype=np.float32).reshape(1, H).astype(bf16)
    wihT = np.ascontiguousarray(
        np.asarray(inputs["Wih"], dtype=np.float32).transpose(0, 2, 1)).astype(bf16)
    whhT = np.ascontiguousarray(
        np.asarray(inputs["Whh"], dtype=np.float32).transpose(0, 2, 1)).astype(bf16)
    bias = np.asarray(inputs["bih"], dtype=np.float32) + np.asarray(inputs["bhh"], dtype=np.float32)
    biasT = np.ascontiguousarray(
        bias.reshape(MAX_SKIP, 8, 128).transpose(2, 0, 1).reshape(128, MAX_SKIP * 8))
    shared = dict(adjT=adjT, rs=rs, wg0=wg0, wg1=wg1, bg0=bg0, bg1=bg1,
                  wihT=wihT, whhT=whhT, biasT=biasT)
    xb = q8(x, XS)
    in_maps = []
    for c in range(NCORES):
        m = dict(shared)
        m["x"] = np.ascontiguousarray(xb[c * BL:(c + 1) * BL])
        in_maps.append(m)
    return in_maps


def get_compiled():
    global _COMPILED
    if _COMPILED is None:
        _COMPILED = _build_program()
    return _COMPILED


def kernel(**inputs) -> np.ndarray:
    from concourse.bass_utils import run_bass_kernel_spmd

    nc = get_compiled()
    in_maps = _prep_host(inputs)
    res = run_bass_kernel_spmd(nc, in_maps, list(range(NCORES)))
    out = np.concatenate([res.results[c]["out"] for c in range(NCORES)], axis=0)
    return out.astype(np.float32)


# revision 27
# speedup vs baseline: 1.0624x; 1.0012x over previous
"""Trainium2 Bass kernel for the DTGL GCN+windowed-LSTM module (bf16 rewrite).

Computation (see reference):
  h = relu(adj @ (x @ Wg0 + bg0));  h = relu(adj @ (h @ Wg1 + bg1))
  for p in 1..4: run LSTM_p over disjoint length-p windows of h (zero init
  state), writing the last hidden state back at each window end (in place).

Sharding: pure data-parallel over batch B=64 across 8 cores (8 batches per
core); adj and all weights replicated. No collectives.

Perf design vs the fp32r baseline:
  - All matmul operands bf16 (PSUM accumulation stays f32): halves the
    moving-stream bytes, halves LDWEIGHTS time, halves SBUF/DMA footprint,
    and lowers PE power so the HAM clock-gate stays at full rate.
  - adjT is SBUF-resident (loaded once, bf16, 64KB/partition).
  - Software-pipelined phases: 1B(j-1) matmuls fill the PE pipe while
    1A(j)'s PSUM->SBUF copies drain (same for 2A/2B), so the PE never
    waits on a copy.
  - LSTM runs batches round-robin per timestep so PE matmuls of batch j+1
    overlap ACT/DVE/Pool cell math of batch j. Gate PSUM uses all 8 banks.
  - Output transpose via identity matmul (stationary = h2T block, moving =
    identity) producing f32 PSUM directly.
"""

import numpy as np

B, T, D, H = 64, 2048, 256, 256
MAX_SKIP = 4
NCORES = 8
BL = B // NCORES          # batches per core
G = 4                     # batches per group
NGRP = BL // G
TK = T // 128             # 16 t-chunks
JB0 = {"i": 0, "f": 2, "g": 4, "o": 6}
XS = 16.0                 # fp8 quantization scales: x, adjT, h1
AS = 8192.0
HS = 64.0

_COMPILED = None


def _build_program():
    import concourse.mybir as mybir
    import concourse.tile as tile
    from concourse import bacc

    f32 = mybir.dt.float32
    bf16 = mybir.dt.bfloat16
    f8 = mybir.dt.float8e4

    nc = bacc.Bacc("TRN2", target_bir_lowering=False, debug=False)

    io = dict(
        x=nc.dram_tensor("x", [BL, T, D], f8, kind="ExternalInput").ap(),
        adjT=nc.dram_tensor("adjT", [T, T], f8, kind="ExternalInput").ap(),
        rs=nc.dram_tensor("rs", [1, T], bf16, kind="ExternalInput").ap(),
        wg0=nc.dram_tensor("wg0", [D, H], bf16, kind="ExternalInput").ap(),
        wg1=nc.dram_tensor("wg1", [D, H], bf16, kind="ExternalInput").ap(),
        bg0=nc.dram_tensor("bg0", [1, H], bf16, kind="ExternalInput").ap(),
        bg1=nc.dram_tensor("bg1", [1, H], bf16, kind="ExternalInput").ap(),
        wihT=nc.dram_tensor("wihT", [MAX_SKIP, H, 4 * H], bf16, kind="ExternalInput").ap(),
        whhT=nc.dram_tensor("whhT", [MAX_SKIP, H, 4 * H], bf16, kind="ExternalInput").ap(),
        biasT=nc.dram_tensor("biasT", [128, MAX_SKIP * 8], f32, kind="ExternalInput").ap(),
        out=nc.dram_tensor("out", [BL, T, D], f32, kind="ExternalOutput").ap(),
    )

    with tile.TileContext(nc) as tc:
        _emit(nc, tc, mybir, io)

    nc.compile()
    return nc


def _emit(nc, tc, mybir, io):
    from contextlib import ExitStack
    from concourse.masks import make_identity

    f32 = mybir.dt.float32
    bf16 = mybir.dt.bfloat16
    f8 = mybir.dt.float8e4
    AF = mybir.ActivationFunctionType

    with ExitStack() as root:
        cp = root.enter_context(tc.tile_pool(name="const", bufs=1))
        # adjT resident: 8 pair-tiles [128, 2*2048] fp8 (ksub-plane-major) for
        # DoubleRow matmuls: plane ks covers rows (2m+ks)*128..(2m+ks+1)*128.
        adjt = []
        for m in range(TK // 2):
            a = cp.tile([128, 2 * T], f8, name=f"adjt_{m}")
            for ks in range(2):
                nc.sync.dma_start(
                    out=a[:, ks * T:(ks + 1) * T],
                    in_=io["adjT"][(2 * m + ks) * 128:(2 * m + ks + 1) * 128, :])
            adjt.append(a)
        wg0_sb = cp.tile([128, 2 * H], bf16, name="wg0_sb")
        wg1_sb = cp.tile([128, 2 * H], bf16, name="wg1_sb")
        for dk in range(2):
            nc.sync.dma_start(out=wg0_sb[:, dk * H:(dk + 1) * H],
                              in_=io["wg0"][dk * 128:(dk + 1) * 128, :])
            nc.sync.dma_start(out=wg1_sb[:, dk * H:(dk + 1) * H],
                              in_=io["wg1"][dk * 128:(dk + 1) * 128, :])
        bg0_sb = cp.tile([1, H], bf16, name="bg0_sb")
        bg1_sb = cp.tile([1, H], bf16, name="bg1_sb")
        rs_sb = cp.tile([1, T], bf16, name="rs_sb")
        biasT_sb = cp.tile([128, MAX_SKIP * 8], f32, name="biasT_sb")
        nc.sync.dma_start(out=bg0_sb[:], in_=io["bg0"][:])
        nc.sync.dma_start(out=bg1_sb[:], in_=io["bg1"][:])
        nc.sync.dma_start(out=rs_sb[:], in_=io["rs"][:])
        nc.sync.dma_start(out=biasT_sb[:], in_=io["biasT"][:])
        id32 = cp.tile([128, 128], f32, name="id32")
        ident = cp.tile([128, 128], bf16, name="ident")
        make_identity(nc, id32[:])
        nc.vector.tensor_copy(ident[:], id32[:])

        h2t_pool = root.enter_context(tc.tile_pool(name="h2tp", bufs=1))
        lw_pool = root.enter_context(tc.tile_pool(name="lw", bufs=2))

        for grp in range(NGRP):
            _group(nc, tc, io, f32, bf16, AF, grp, adjt, wg0_sb, wg1_sb,
                   bg0_sb, bg1_sb, rs_sb, biasT_sb, ident, h2t_pool, lw_pool)


def _group(nc, tc, io, f32, bf16, AF, grp, adjt, wg0_sb, wg1_sb, bg0_sb,
           bg1_sb, rs_sb, biasT_sb, ident, h2t_pool, lw_pool):
    from contextlib import ExitStack
    import concourse.mybir as mybir

    f8 = mybir.dt.float8e4
    DR = mybir.MatmulPerfMode.DoubleRow
    bs = grp * G
    # h2T slabs: feature-major [h(part within hk), hk*T + t], bf16.
    h2t = [h2t_pool.tile([128, 2 * T], bf16, name=f"h2t_{j}", tag=f"h2t_{j}")
           for j in range(G)]

    with ExitStack() as gcn:
        h1_pool = gcn.enter_context(tc.tile_pool(name="h1p", bufs=1))
        # h1 slabs: row-major [u(part within ub), ub*H + h], fp8 scaled by HS.
        h1 = [h1_pool.tile([128, TK * H], f8, name=f"h1_{j}", tag=f"h1_{j}")
              for j in range(G)]
        x_pool = gcn.enter_context(tc.tile_pool(name="xp", bufs=2))
        z1_pool = gcn.enter_context(tc.tile_pool(name="z1p", bufs=2))

        # ---------------- Phase 1: layer 1 (1A + 1B pipelined) ----------------
        with ExitStack() as ph:
            zps = ph.enter_context(tc.tile_pool(name="zps", bufs=1, space="PSUM"))
            hps = ph.enter_context(tc.tile_pool(name="hps", bufs=4, space="PSUM"))

            xs = []
            for j in range(G):
                xt = x_pool.tile([128, TK * D], f8, name=f"x_{j}", tag="xs")
                nc.sync.dma_start(
                    out=xt[:].rearrange("p (k d) -> p k d", d=D),
                    in_=io["x"][bs + j].rearrange("(k p) d -> p k d", p=128))
                xs.append(xt)

            z1t = {}   # (j, uh, dk) -> sbuf tile [128, 1024] bf16
            pend = []  # queue of emitted-1A halves awaiting 1B: (j, uh)

            def emit_1a(j, uh):
                zp = {(dk, q): zps.tile([128, 512], f32, name="zp", tag=f"zp{dk}{q}")
                      for dk in range(2) for q in range(2)}
                xv = xs[j][:].rearrange("p (k d) -> p k d", d=D)
                for m in range(TK // 2):
                    av = adjt[m][:].rearrange("p (k u) -> p k u", k=2)
                    for dk in range(2):
                        lhs = xv[:, 2 * m:2 * m + 2, dk * 128:(dk + 1) * 128]
                        for q in range(2):
                            us = uh * 1024 + q * 512
                            nc.tensor.matmul(
                                zp[(dk, q)][:], lhs, av[:, :, us:us + 512],
                                start=(m == 0), stop=(m == TK // 2 - 1),
                                perf_mode=DR)
                for dk in range(2):
                    zt = z1_pool.tile([128, 1024], bf16, name="z1t", tag=f"z1t{dk}")
                    z1t[(j, uh, dk)] = zt
                    nc.vector.tensor_scalar_mul(zt[:, 0:512], zp[(dk, 0)][:], 1.0 / (XS * AS))
                    nc.vector.tensor_scalar_mul(zt[:, 512:1024], zp[(dk, 1)][:], 1.0 / (XS * AS))

            def emit_1b(j, uh):
                for ub_l in range(8):
                    ub = uh * 8 + ub_l
                    hp = hps.tile([128, H], f32, name="hp", tag="hp")
                    for dk in range(2):
                        nc.tensor.matmul(
                            hp[:], z1t[(j, uh, dk)][:, ub_l * 128:(ub_l + 1) * 128],
                            wg0_sb[:, dk * H:(dk + 1) * H],
                            start=(dk == 0), stop=False)
                    nc.tensor.matmul(hp[:], rs_sb[0:1, ub * 128:(ub + 1) * 128],
                                     bg0_sb[0:1, :], start=False, stop=True)
                    # h1 = relu(HS * z) in fp8 (scale folded through relu)
                    nc.scalar.activation(h1[j][:, ub * H:(ub + 1) * H], hp[:],
                                         AF.Relu, scale=HS)

            for j in range(G):
                for uh in range(2):
                    emit_1a(j, uh)
                    pend.append((j, uh))
                    if len(pend) > 1:
                        emit_1b(*pend.pop(0))
            while pend:
                emit_1b(*pend.pop(0))

        # ---------------- Phase 2: layer 2 (2A + 2B pipelined) ----------------
        with ExitStack() as ph:
            zps = ph.enter_context(tc.tile_pool(name="zps2", bufs=1, space="PSUM"))
            hps = ph.enter_context(tc.tile_pool(name="hps2", bufs=1, space="PSUM"))
            z2_pool = ph.enter_context(tc.tile_pool(name="z2p", bufs=2))

            z2t = {}
            pend = []

            def emit_2a(j, uh):
                zp = {(hk, q): zps.tile([128, 512], f32, name="zp2", tag=f"zp2{hk}{q}")
                      for hk in range(2) for q in range(2)}
                hv = h1[j][:].rearrange("p (u h) -> p u h", h=H)
                for m in range(TK // 2):
                    av = adjt[m][:].rearrange("p (k u) -> p k u", k=2)
                    for hk in range(2):
                        lhs = hv[:, 2 * m:2 * m + 2, hk * 128:(hk + 1) * 128]
                        for q in range(2):
                            us = uh * 1024 + q * 512
                            nc.tensor.matmul(
                                zp[(hk, q)][:], lhs, av[:, :, us:us + 512],
                                start=(m == 0), stop=(m == TK // 2 - 1),
                                perf_mode=DR)
                for hk in range(2):
                    zt = z2_pool.tile([128, 1024], bf16, name="z2t", tag=f"z2t{hk}")
                    z2t[(j, uh, hk)] = zt
                    nc.vector.tensor_scalar_mul(zt[:, 0:512], zp[(hk, 0)][:], 1.0 / (HS * AS))
                    nc.vector.tensor_scalar_mul(zt[:, 512:1024], zp[(hk, 1)][:], 1.0 / (HS * AS))

            def emit_2b(j, uh):
                for ho in range(2):
                    for q in range(2):
                        hp = hps.tile([128, 512], f32, name="hp2", tag=f"hp2{ho}{q}")
                        for hk in range(2):
                            nc.tensor.matmul(
                                hp[:], wg1_sb[:, hk * H + ho * 128: hk * H + (ho + 1) * 128],
                                z2t[(j, uh, hk)][:, q * 512:(q + 1) * 512],
                                start=(hk == 0), stop=False)
                        us = uh * 1024 + q * 512
                        nc.tensor.matmul(hp[:], bg1_sb[0:1, ho * 128:(ho + 1) * 128],
                                         rs_sb[0:1, us:us + 512], start=False, stop=True)
                        nc.vector.tensor_relu(h2t[j][:, ho * T + us: ho * T + us + 512],
                                              hp[:])

            for j in range(G):
                for uh in range(2):
                    emit_2a(j, uh)
                    pend.append((j, uh))
                    if len(pend) > 1:
                        emit_2b(*pend.pop(0))
            while pend:
                emit_2b(*pend.pop(0))

    # ---------------- Phases 3-4: the four LSTM passes ----------------
    # Two cohorts (j0,j1 | j2,j3) run with a one-pass skew so ACT-heavy t=0
    # units (esp. all of p=1) interleave with PE-heavy t>0 units.
    with ExitStack() as ph:
        gps = ph.enter_context(tc.tile_pool(name="gps", bufs=1, space="PSUM"))
        gsb = ph.enter_context(tc.tile_pool(name="gsb", bufs=3))
        st_pool = ph.enter_context(tc.tile_pool(name="st", bufs=1))
        h_pool = ph.enter_context(tc.tile_pool(name="hs", bufs=2))
        gx_pool = ph.enter_context(tc.tile_pool(name="gx", bufs=3))

        c_st = [st_pool.tile([128, 1024], bf16, name=f"c_{j}", tag=f"c{j}")
                for j in range(G)]
        h_t = [None] * G
        lw = {}

        def get_weights(p):
            if p in lw:
                return lw[p]
            wih = lw_pool.tile([128, 2 * 4 * H], bf16, name=f"wih{grp}{p}", tag="wih")
            for hk in range(2):
                nc.sync.dma_start(out=wih[:, hk * 4 * H:(hk + 1) * 4 * H],
                                  in_=io["wihT"][p - 1, hk * 128:(hk + 1) * 128, :])
            whh = None
            if p > 1:
                whh = lw_pool.tile([128, 2 * 4 * H], bf16, name=f"whh{grp}{p}", tag="whh")
                for hk in range(2):
                    nc.sync.dma_start(out=whh[:, hk * 4 * H:(hk + 1) * 4 * H],
                                      in_=io["whhT"][p - 1, hk * 128:(hk + 1) * 128, :])
            lw[p] = (wih, whh)
            return lw[p]

        def _mkview(p, j):
            nw = T // p
            return [h2t[j][:, hk * T: hk * T + nw * p].rearrange(
                "a (w q) -> a w q", q=p) for hk in range(2)]

        def emit_gather(p, ws, t, j):
            if p == 1:
                return None
            nw = T // p
            ncw = min(512, nw - ws)
            view = _mkview(p, j)
            xc = gx_pool.tile([128, 1024], bf16, name="xc", tag="xc")
            nc.gpsimd.tensor_copy(xc[:, 0:ncw],
                                  view[0][:, ws:ws + ncw, t:t + 1])
            nc.vector.tensor_copy(xc[:, 512:512 + ncw],
                                  view[1][:, ws:ws + ncw, t:t + 1])
            return xc

        def emit_head(p, ws, t, j, xc):
            nw = T // p
            ncw = min(512, nw - ws)
            spans = ([slice(0, 1024)] if ncw == 512
                     else [slice(0, ncw), slice(512, 512 + ncw)])
            wih, whh = get_weights(p)
            view = _mkview(p, j)
            if True:
                    if True:
                        gates = "igo" if t == 0 else "ifgo"
                        gp = {}
                        # input-weight matmuls first (no state dependency)
                        for gn in gates:
                            psum = gps.tile([128, 1024], f32, name=f"ps_{gn}", tag=f"ps_{gn}")
                            gp[gn] = psum
                            for half in range(2):
                                jb = JB0[gn] + half
                                o = psum[:, half * 512: half * 512 + ncw]
                                for hk in range(2):
                                    rhs = (view[hk][:, ws:ws + ncw, 0:1] if p == 1
                                           else xc[:, hk * 512: hk * 512 + ncw])
                                    nc.tensor.matmul(
                                        o,
                                        wih[:, hk * 4 * H + jb * 128: hk * 4 * H + (jb + 1) * 128],
                                        rhs,
                                        start=(hk == 0),
                                        stop=(t == 0 and hk == 1))
                        if t > 0:
                            for gn in gates:
                                for half in range(2):
                                    jb = JB0[gn] + half
                                    o = gp[gn][:, half * 512: half * 512 + ncw]
                                    for hk in range(2):
                                        nc.tensor.matmul(
                                            o,
                                            whh[:, hk * 4 * H + jb * 128: hk * 4 * H + (jb + 1) * 128],
                                            h_t[j][:, hk * 512: hk * 512 + ncw],
                                            start=False, stop=(hk == 1))
                        act = {}
                        for gn in gates:
                            fn = AF.Tanh if gn == "g" else AF.Sigmoid
                            a = gsb.tile([128, 1024], bf16, name=f"a_{gn}", tag=f"a_{gn}")
                            act[gn] = a
                            for half in range(2):
                                col = (p - 1) * 8 + JB0[gn] + half
                                nc.scalar.activation(
                                    a[:, half * 512: half * 512 + ncw],
                                    gp[gn][:, half * 512: half * 512 + ncw],
                                    fn, bias=biasT_sb[:, col:col + 1])
                        cn = c_st[j]
                        if t == 0:
                            for s in spans:
                                nc.vector.tensor_mul(cn[:, s], act["i"][:, s], act["g"][:, s])
                        else:
                            for s in spans:
                                nc.vector.tensor_mul(act["g"][:, s], act["i"][:, s], act["g"][:, s])
                            for s in spans:
                                nc.gpsimd.tensor_mul(cn[:, s], act["f"][:, s], cn[:, s])
                            for s in spans:
                                nc.vector.tensor_add(cn[:, s], cn[:, s], act["g"][:, s])
                        # tail (tanh + h-mul) deferred one unit to avoid ACT
                        # head-of-line blocking on the not-yet-ready c input
                        return (p, ws, t, j, ncw, spans, view, act, cn)

        def emit_tail(p, ws, t, j, ncw, spans, view, act, cn):
            # tanh(c) overwrites the i tile (free after c update)
            tc_t = act["i"]
            for s in spans:
                nc.scalar.activation(tc_t[:, s], cn[:, s], AF.Tanh)
            if t == p - 1:
                for hk in range(2):
                    nc.vector.tensor_mul(
                        view[hk][:, ws:ws + ncw, p - 1:p],
                        act["o"][:, hk * 512: hk * 512 + ncw],
                        tc_t[:, hk * 512: hk * 512 + ncw])
            else:
                hn = h_pool.tile([128, 1024], bf16, name="hn", tag=f"h{j}")
                for s in spans:
                    nc.vector.tensor_mul(hn[:, s], act["o"][:, s], tc_t[:, s])
                h_t[j] = hn

        units = []
        for p in range(1, MAX_SKIP + 1):
            nw = T // p
            for ws in range(0, nw, 512):
                for t in range(p):
                    for j in range(G):
                        units.append((p, ws, t, j))

        pend_tail = []
        xcs = {}
        xcs[0] = emit_gather(*units[0])
        for n, u in enumerate(units):
            if n + 1 < len(units):
                xcs[n + 1] = emit_gather(*units[n + 1])
            tail = emit_head(*u, xcs.pop(n))
            pend_tail.append(tail)
            if len(pend_tail) > 1:
                emit_tail(*pend_tail.pop(0))
        while pend_tail:
            emit_tail(*pend_tail.pop(0))

        # ------------- Phase 5: transpose h2T -> out (reuses gate PSUM) -------------
        osb = ph.enter_context(tc.tile_pool(name="osb", bufs=2))
        tptags = ["ps_i", "ps_f", "ps_g", "ps_o"]
        for j in range(G):
            b = bs + j
            for tg in range(4):
                tp = gps.tile([128, 1024], f32, name="tp", tag=tptags[tg])
                for q in range(4):
                    tk = tg * 4 + q
                    for hk in range(2):
                        nc.tensor.matmul(
                            tp[:, q * D + hk * 128: q * D + (hk + 1) * 128],
                            h2t[j][:, hk * T + tk * 128: hk * T + (tk + 1) * 128],
                            ident[:], start=True, stop=True)
                ot = osb.tile([128, 1024], f32, name="ot", tag="ot")
                if tg % 2 == 0:
                    nc.scalar.activation(ot[:], tp[:], AF.Copy)
                else:
                    nc.vector.tensor_copy(ot[:], tp[:])
                nc.sync.dma_start(
                    out=io["out"][b, tg * 512:(tg + 1) * 512, :].rearrange(
                        "(q p) d -> p q d", p=128),
                    in_=ot[:].rearrange("p (q d) -> p q d", d=D))


def _prep_host(inputs):
    import ml_dtypes
    bf16 = ml_dtypes.bfloat16
    f8 = ml_dtypes.float8_e4m3fn

    def q8(a, scale):
        return np.ascontiguousarray(
            np.clip(np.asarray(a, dtype=np.float32) * scale, -240, 240)).astype(f8)

    x = np.asarray(inputs["x"], dtype=np.float32)
    adj = np.asarray(inputs["adj"], dtype=np.float32)
    adjT = q8(adj.T, AS)
    rs = np.ascontiguousarray(
        adj.sum(axis=1, dtype=np.float32).reshape(1, T)).astype(bf16)
    wg0 = np.ascontiguousarray(inputs["Wg0"], dtype=np.float32).astype(bf16)
    wg1 = np.ascontiguousarray(inputs["Wg1"], dtype=np.float32).astype(bf16)
    bg0 = np.ascontiguousarray(inputs["bg0"], dtype=np.float32).reshape(1, H).astype(bf16)
    bg1 = np.ascontiguousarray(inputs["bg1"], dtype=np.float32).reshape(1, H).astype(bf16)
    wihT = np.ascontiguousarray(
        np.asarray(inputs["Wih"], dtype=np.float32).transpose(0, 2, 1)).astype(bf16)
    whhT = np.ascontiguousarray(
        np.asarray(inputs["Whh"], dtype=np.float32).transpose(0, 2, 1)).astype(bf16)
    bias = np.asarray(inputs["bih"], dtype=np.float32) + np.asarray(inputs["bhh"], dtype=np.float32)
    biasT = np.ascontiguousarray(
        bias.reshape(MAX_SKIP, 8, 128).transpose(2, 0, 1).reshape(128, MAX_SKIP * 8))
    shared = dict(adjT=adjT, rs=rs, wg0=wg0, wg1=wg1, bg0=bg0, bg1=bg1,
                  wihT=wihT, whhT=whhT, biasT=biasT)
    xb = q8(x, XS)
    in_maps = []
    for c in range(NCORES):
        m = dict(shared)
        m["x"] = np.ascontiguousarray(xb[c * BL:(c + 1) * BL])
        in_maps.append(m)
    return in_maps


def get_compiled():
    global _COMPILED
    if _COMPILED is None:
        _COMPILED = _build_program()
    return _COMPILED


def kernel(**inputs) -> np.ndarray:
    from concourse.bass_utils import run_bass_kernel_spmd

    nc = get_compiled()
    in_maps = _prep_host(inputs)
    res = run_bass_kernel_spmd(nc, in_maps, list(range(NCORES)))
    out = np.concatenate([res.results[c]["out"] for c in range(NCORES)], axis=0)
    return out.astype(np.float32)


# revision 32
# speedup vs baseline: 1.0825x; 1.0189x over previous
"""Trainium2 Bass kernel for the DTGL GCN+windowed-LSTM module (bf16 rewrite).

Computation (see reference):
  h = relu(adj @ (x @ Wg0 + bg0));  h = relu(adj @ (h @ Wg1 + bg1))
  for p in 1..4: run LSTM_p over disjoint length-p windows of h (zero init
  state), writing the last hidden state back at each window end (in place).

Sharding: pure data-parallel over batch B=64 across 8 cores (8 batches per
core); adj and all weights replicated. No collectives.

Perf design vs the fp32r baseline:
  - All matmul operands bf16 (PSUM accumulation stays f32): halves the
    moving-stream bytes, halves LDWEIGHTS time, halves SBUF/DMA footprint,
    and lowers PE power so the HAM clock-gate stays at full rate.
  - adjT is SBUF-resident (loaded once, bf16, 64KB/partition).
  - Software-pipelined phases: 1B(j-1) matmuls fill the PE pipe while
    1A(j)'s PSUM->SBUF copies drain (same for 2A/2B), so the PE never
    waits on a copy.
  - LSTM runs batches round-robin per timestep so PE matmuls of batch j+1
    overlap ACT/DVE/Pool cell math of batch j. Gate PSUM uses all 8 banks.
  - Output transpose via identity matmul (stationary = h2T block, moving =
    identity) producing f32 PSUM directly.
"""

import numpy as np

B, T, D, H = 64, 2048, 256, 256
MAX_SKIP = 4
NCORES = 8
BL = B // NCORES          # batches per core
G = 4                     # batches per group
NGRP = BL // G
TK = T // 128             # 16 t-chunks
JB0 = {"i": 0, "f": 2, "g": 4, "o": 6}
XS = 16.0                 # fp8 quantization scales: x, adjT, h1
AS = 8192.0
HS = 64.0

_COMPILED = None


def _build_program():
    import concourse.mybir as mybir
    import concourse.tile as tile
    from concourse import bacc

    f32 = mybir.dt.float32
    bf16 = mybir.dt.bfloat16
    f8 = mybir.dt.float8e4

    nc = bacc.Bacc("TRN2", target_bir_lowering=False, debug=False)

    io = dict(
        x=nc.dram_tensor("x", [BL, T, D], f8, kind="ExternalInput").ap(),
        adjT=nc.dram_tensor("adjT", [T, T], f8, kind="ExternalInput").ap(),
        rs=nc.dram_tensor("rs", [1, T], bf16, kind="ExternalInput").ap(),
        wg0=nc.dram_tensor("wg0", [D, H], bf16, kind="ExternalInput").ap(),
        wg1=nc.dram_tensor("wg1", [D, H], bf16, kind="ExternalInput").ap(),
        bg0=nc.dram_tensor("bg0", [1, H], bf16, kind="ExternalInput").ap(),
        bg1=nc.dram_tensor("bg1", [1, H], bf16, kind="ExternalInput").ap(),
        wihT=nc.dram_tensor("wihT", [MAX_SKIP, H, 4 * H], bf16, kind="ExternalInput").ap(),
        whhT=nc.dram_tensor("whhT", [MAX_SKIP, H, 4 * H], bf16, kind="ExternalInput").ap(),
        biasT=nc.dram_tensor("biasT", [128, MAX_SKIP * 8], f32, kind="ExternalInput").ap(),
        out=nc.dram_tensor("out", [BL, T, D], f32, kind="ExternalOutput").ap(),
    )

    with tile.TileContext(nc) as tc:
        _emit(nc, tc, mybir, io)

    nc.compile()
    return nc


def _emit(nc, tc, mybir, io):
    from contextlib import ExitStack
    from concourse.masks import make_identity

    f32 = mybir.dt.float32
    bf16 = mybir.dt.bfloat16
    f8 = mybir.dt.float8e4
    AF = mybir.ActivationFunctionType

    with ExitStack() as root:
        cp = root.enter_context(tc.tile_pool(name="const", bufs=1))
        # adjT resident: 8 pair-tiles [128, 2*2048] fp8 (ksub-plane-major) for
        # DoubleRow matmuls: plane ks covers rows (2m+ks)*128..(2m+ks+1)*128.
        adjt = []
        for m in range(TK // 2):
            a = cp.tile([128, 2 * T], f8, name=f"adjt_{m}")
            for ks in range(2):
                nc.sync.dma_start(
                    out=a[:, ks * T:(ks + 1) * T],
                    in_=io["adjT"][(2 * m + ks) * 128:(2 * m + ks + 1) * 128, :])
            adjt.append(a)
        wg0_sb = cp.tile([128, 2 * H], bf16, name="wg0_sb")
        wg1_sb = cp.tile([128, 2 * H], bf16, name="wg1_sb")
        for dk in range(2):
            nc.sync.dma_start(out=wg0_sb[:, dk * H:(dk + 1) * H],
                              in_=io["wg0"][dk * 128:(dk + 1) * 128, :])
            nc.sync.dma_start(out=wg1_sb[:, dk * H:(dk + 1) * H],
                              in_=io["wg1"][dk * 128:(dk + 1) * 128, :])
        bg0_sb = cp.tile([1, H], bf16, name="bg0_sb")
        bg1_sb = cp.tile([1, H], bf16, name="bg1_sb")
        rs_sb = cp.tile([1, T], bf16, name="rs_sb")
        biasT_sb = cp.tile([128, MAX_SKIP * 8], f32, name="biasT_sb")
        nc.sync.dma_start(out=bg0_sb[:], in_=io["bg0"][:])
        nc.sync.dma_start(out=bg1_sb[:], in_=io["bg1"][:])
        nc.sync.dma_start(out=rs_sb[:], in_=io["rs"][:])
        nc.sync.dma_start(out=biasT_sb[:], in_=io["biasT"][:])
        id32 = cp.tile([128, 128], f32, name="id32")
        ident = cp.tile([128, 128], bf16, name="ident")
        make_identity(nc, id32[:])
        nc.vector.tensor_copy(ident[:], id32[:])

        h2t_pool = root.enter_context(tc.tile_pool(name="h2tp", bufs=1))
        lw_pool = root.enter_context(tc.tile_pool(name="lw", bufs=2))

        # all x tiles upfront: [128(t within k-block), k*D + d] fp8, one per batch
        xs_all = []
        for b in range(BL):
            xt = h2t_pool.tile([128, TK * D], f8, name=f"x_{b}", tag=f"x_{b}")
            nc.sync.dma_start(
                out=xt[:].rearrange("p (k d) -> p k d", d=D),
                in_=io["x"][b].rearrange("(k p) d -> p k d", p=128))
            xs_all.append(xt)

        for grp in range(NGRP):
            _group(nc, tc, io, f32, bf16, AF, grp, adjt, wg0_sb, wg1_sb,
                   bg0_sb, bg1_sb, rs_sb, biasT_sb, ident, h2t_pool, lw_pool,
                   xs_all)


def _group(nc, tc, io, f32, bf16, AF, grp, adjt, wg0_sb, wg1_sb, bg0_sb,
           bg1_sb, rs_sb, biasT_sb, ident, h2t_pool, lw_pool, xs_all):
    from contextlib import ExitStack
    import concourse.mybir as mybir

    f8 = mybir.dt.float8e4
    DR = mybir.MatmulPerfMode.DoubleRow
    bs = grp * G
    # h2T slabs: feature-major [h(part within hk), hk*T + t], bf16.
    h2t = [h2t_pool.tile([128, 2 * T], bf16, name=f"h2t_{j}", tag=f"h2t_{j}")
           for j in range(G)]

    with ExitStack() as gcn:
        h1_pool = gcn.enter_context(tc.tile_pool(name="h1p", bufs=1))
        # h1 slabs: row-major [u(part within ub), ub*H + h], fp8 scaled by HS.
        h1 = [h1_pool.tile([128, TK * H], f8, name=f"h1_{j}", tag=f"h1_{j}")
              for j in range(G)]
        z1_pool = gcn.enter_context(tc.tile_pool(name="z1p", bufs=2))

        # ---------------- Phase 1: layer 1 (1A + 1B pipelined) ----------------
        with ExitStack() as ph:
            zps = ph.enter_context(tc.tile_pool(name="zps", bufs=1, space="PSUM"))
            hps = ph.enter_context(tc.tile_pool(name="hps", bufs=4, space="PSUM"))

            xs = [xs_all[bs + j] for j in range(G)]

            z1t = {}   # (j, uh, dk) -> sbuf tile [128, 1024] bf16
            pend = []  # queue of emitted-1A halves awaiting 1B: (j, uh)

            def emit_1a(j, uh):
                zp = {(dk, q): zps.tile([128, 512], f32, name="zp", tag=f"zp{dk}{q}")
                      for dk in range(2) for q in range(2)}
                xv = xs[j][:].rearrange("p (k d) -> p k d", d=D)
                for m in range(TK // 2):
                    av = adjt[m][:].rearrange("p (k u) -> p k u", k=2)
                    for dk in range(2):
                        lhs = xv[:, 2 * m:2 * m + 2, dk * 128:(dk + 1) * 128]
                        for q in range(2):
                            us = uh * 1024 + q * 512
                            nc.tensor.matmul(
                                zp[(dk, q)][:], lhs, av[:, :, us:us + 512],
                                start=(m == 0), stop=(m == TK // 2 - 1),
                                perf_mode=DR)
                for dk in range(2):
                    zt = z1_pool.tile([128, 1024], bf16, name="z1t", tag=f"z1t{dk}")
                    z1t[(j, uh, dk)] = zt
                    nc.vector.tensor_scalar_mul(zt[:, 0:512], zp[(dk, 0)][:], 1.0 / (XS * AS))
                    nc.vector.tensor_scalar_mul(zt[:, 512:1024], zp[(dk, 1)][:], 1.0 / (XS * AS))

            def emit_1b(j, uh):
                for ub_l in range(8):
                    ub = uh * 8 + ub_l
                    hp = hps.tile([128, H], f32, name="hp", tag="hp")
                    for dk in range(2):
                        nc.tensor.matmul(
                            hp[:], z1t[(j, uh, dk)][:, ub_l * 128:(ub_l + 1) * 128],
                            wg0_sb[:, dk * H:(dk + 1) * H],
                            start=(dk == 0), stop=False)
                    nc.tensor.matmul(hp[:], rs_sb[0:1, ub * 128:(ub + 1) * 128],
                                     bg0_sb[0:1, :], start=False, stop=True)
                    # h1 = relu(HS * z) in fp8 (scale folded through relu)
                    nc.scalar.activation(h1[j][:, ub * H:(ub + 1) * H], hp[:],
                                         AF.Relu, scale=HS)

            for j in range(G):
                for uh in range(2):
                    emit_1a(j, uh)
                    pend.append((j, uh))
                    if len(pend) > 1:
                        emit_1b(*pend.pop(0))
            while pend:
                emit_1b(*pend.pop(0))

        # ---------------- Phase 2: layer 2 (2A + 2B pipelined) ----------------
        with ExitStack() as ph:
            zps = ph.enter_context(tc.tile_pool(name="zps2", bufs=1, space="PSUM"))
            hps = ph.enter_context(tc.tile_pool(name="hps2", bufs=1, space="PSUM"))
            z2_pool = ph.enter_context(tc.tile_pool(name="z2p", bufs=2))

            z2t = {}
            pend = []

            def emit_2a(j, uh):
                zp = {(hk, q): zps.tile([128, 512], f32, name="zp2", tag=f"zp2{hk}{q}")
                      for hk in range(2) for q in range(2)}
                hv = h1[j][:].rearrange("p (u h) -> p u h", h=H)
                for m in range(TK // 2):
                    av = adjt[m][:].rearrange("p (k u) -> p k u", k=2)
                    for hk in range(2):
                        lhs = hv[:, 2 * m:2 * m + 2, hk * 128:(hk + 1) * 128]
                        for q in range(2):
                            us = uh * 1024 + q * 512
                            nc.tensor.matmul(
                                zp[(hk, q)][:], lhs, av[:, :, us:us + 512],
                                start=(m == 0), stop=(m == TK // 2 - 1),
                                perf_mode=DR)
                for hk in range(2):
                    zt = z2_pool.tile([128, 1024], bf16, name="z2t", tag=f"z2t{hk}")
                    z2t[(j, uh, hk)] = zt
                    nc.vector.tensor_scalar_mul(zt[:, 0:512], zp[(hk, 0)][:], 1.0 / (HS * AS))
                    nc.vector.tensor_scalar_mul(zt[:, 512:1024], zp[(hk, 1)][:], 1.0 / (HS * AS))

            def emit_2b(j, uh):
                for ho in range(2):
                    for q in range(2):
                        hp = hps.tile([128, 512], f32, name="hp2", tag=f"hp2{ho}{q}")
                        for hk in range(2):
                            nc.tensor.matmul(
                                hp[:], wg1_sb[:, hk * H + ho * 128: hk * H + (ho + 1) * 128],
                                z2t[(j, uh, hk)][:, q * 512:(q + 1) * 512],
                                start=(hk == 0), stop=False)
                        us = uh * 1024 + q * 512
                        nc.tensor.matmul(hp[:], bg1_sb[0:1, ho * 128:(ho + 1) * 128],
                                         rs_sb[0:1, us:us + 512], start=False, stop=True)
                        nc.vector.tensor_relu(h2t[j][:, ho * T + us: ho * T + us + 512],
                                              hp[:])

            for j in range(G):
                for uh in range(2):
                    emit_2a(j, uh)
                    pend.append((j, uh))
                    if len(pend) > 1:
                        emit_2b(*pend.pop(0))
            while pend:
                emit_2b(*pend.pop(0))

    # ---------------- Phases 3-4: the four LSTM passes ----------------
    # Two cohorts (j0,j1 | j2,j3) run with a one-pass skew so ACT-heavy t=0
    # units (esp. all of p=1) interleave with PE-heavy t>0 units.
    with ExitStack() as ph:
        gps = ph.enter_context(tc.tile_pool(name="gps", bufs=1, space="PSUM"))
        gsb = ph.enter_context(tc.tile_pool(name="gsb", bufs=3))
        st_pool = ph.enter_context(tc.tile_pool(name="st", bufs=1))
        h_pool = ph.enter_context(tc.tile_pool(name="hs", bufs=2))
        gx_pool = ph.enter_context(tc.tile_pool(name="gx", bufs=3))

        c_st = [st_pool.tile([128, 1024], bf16, name=f"c_{j}", tag=f"c{j}")
                for j in range(G)]
        h_t = [None] * G
        lw = {}

        def get_weights(p):
            if p in lw:
                return lw[p]
            wih = lw_pool.tile([128, 2 * 4 * H], bf16, name=f"wih{grp}{p}", tag="wih")
            for hk in range(2):
                nc.sync.dma_start(out=wih[:, hk * 4 * H:(hk + 1) * 4 * H],
                                  in_=io["wihT"][p - 1, hk * 128:(hk + 1) * 128, :])
            whh = None
            if p > 1:
                whh = lw_pool.tile([128, 2 * 4 * H], bf16, name=f"whh{grp}{p}", tag="whh")
                for hk in range(2):
                    nc.sync.dma_start(out=whh[:, hk * 4 * H:(hk + 1) * 4 * H],
                                      in_=io["whhT"][p - 1, hk * 128:(hk + 1) * 128, :])
            lw[p] = (wih, whh)
            return lw[p]

        def _mkview(p, j):
            nw = T // p
            return [h2t[j][:, hk * T: hk * T + nw * p].rearrange(
                "a (w q) -> a w q", q=p) for hk in range(2)]

        def emit_gather(p, ws, t, j):
            if p == 1:
                return None
            nw = T // p
            ncw = min(512, nw - ws)
            view = _mkview(p, j)
            xc = gx_pool.tile([128, 1024], bf16, name="xc", tag="xc")
            nc.gpsimd.tensor_copy(xc[:, 0:ncw],
                                  view[0][:, ws:ws + ncw, t:t + 1])
            nc.vector.tensor_copy(xc[:, 512:512 + ncw],
                                  view[1][:, ws:ws + ncw, t:t + 1])
            return xc

        def emit_head(p, ws, t, j, xc):
            nw = T // p
            ncw = min(512, nw - ws)
            spans = ([slice(0, 1024)] if ncw == 512
                     else [slice(0, ncw), slice(512, 512 + ncw)])
            wih, whh = get_weights(p)
            view = _mkview(p, j)
            if True:
                    if True:
                        gates = "igo" if t == 0 else "ifgo"
                        gp = {}
                        # input-weight matmuls first (no state dependency)
                        for gn in gates:
                            psum = gps.tile([128, 1024], f32, name=f"ps_{gn}", tag=f"ps_{gn}")
                            gp[gn] = psum
                            for half in range(2):
                                jb = JB0[gn] + half
                                o = psum[:, half * 512: half * 512 + ncw]
                                for hk in range(2):
                                    rhs = (view[hk][:, ws:ws + ncw, 0:1] if p == 1
                                           else xc[:, hk * 512: hk * 512 + ncw])
                                    nc.tensor.matmul(
                                        o,
                                        wih[:, hk * 4 * H + jb * 128: hk * 4 * H + (jb + 1) * 128],
                                        rhs,
                                        start=(hk == 0),
                                        stop=(t == 0 and hk == 1))
                        if t > 0:
                            for gn in gates:
                                for half in range(2):
                                    jb = JB0[gn] + half
                                    o = gp[gn][:, half * 512: half * 512 + ncw]
                                    for hk in range(2):
                                        nc.tensor.matmul(
                                            o,
                                            whh[:, hk * 4 * H + jb * 128: hk * 4 * H + (jb + 1) * 128],
                                            h_t[j][:, hk * 512: hk * 512 + ncw],
                                            start=False, stop=(hk == 1))
                        act = {}
                        for gn in gates:
                            fn = AF.Tanh if gn == "g" else AF.Sigmoid
                            a = gsb.tile([128, 1024], bf16, name=f"a_{gn}", tag=f"a_{gn}")
                            act[gn] = a
                            for half in range(2):
                                col = (p - 1) * 8 + JB0[gn] + half
                                nc.scalar.activation(
                                    a[:, half * 512: half * 512 + ncw],
                                    gp[gn][:, half * 512: half * 512 + ncw],
                                    fn, bias=biasT_sb[:, col:col + 1])
                        cn = c_st[j]
                        if t == 0:
                            for s in spans:
                                nc.vector.tensor_mul(cn[:, s], act["i"][:, s], act["g"][:, s])
                        else:
                            for s in spans:
                                nc.vector.tensor_mul(act["g"][:, s], act["i"][:, s], act["g"][:, s])
                            for s in spans:
                                nc.gpsimd.tensor_mul(cn[:, s], act["f"][:, s], cn[:, s])
                            for s in spans:
                                nc.vector.tensor_add(cn[:, s], cn[:, s], act["g"][:, s])
                        # tail (tanh + h-mul) deferred one unit to avoid ACT
                        # head-of-line blocking on the not-yet-ready c input
                        return (p, ws, t, j, ncw, spans, view, act, cn)

        def emit_tail(p, ws, t, j, ncw, spans, view, act, cn):
            # tanh(c) overwrites the i tile (free after c update)
            tc_t = act["i"]
            for s in spans:
                nc.scalar.activation(tc_t[:, s], cn[:, s], AF.Tanh)
            if t == p - 1:
                for hk in range(2):
                    nc.vector.tensor_mul(
                        view[hk][:, ws:ws + ncw, p - 1:p],
                        act["o"][:, hk * 512: hk * 512 + ncw],
                        tc_t[:, hk * 512: hk * 512 + ncw])
            else:
                hn = h_pool.tile([128, 1024], bf16, name="hn", tag=f"h{j}")
                for s in spans:
                    nc.vector.tensor_mul(hn[:, s], act["o"][:, s], tc_t[:, s])
                h_t[j] = hn

        # ---- transpose-out emitter (phase 5), interleaved into the p=4 tail ----
        osb = ph.enter_context(tc.tile_pool(name="osb", bufs=2))
        tptags = ["ps_i", "ps_f", "ps_g", "ps_o"]

        def emit_transpose(j):
            b = bs + j
            for tg in range(4):
                tp = gps.tile([128, 1024], f32, name="tp", tag=tptags[tg])
                for q in range(4):
                    tk = tg * 4 + q
                    for hk in range(2):
                        nc.tensor.matmul(
                            tp[:, q * D + hk * 128: q * D + (hk + 1) * 128],
                            h2t[j][:, hk * T + tk * 128: hk * T + (tk + 1) * 128],
                            ident[:], start=True, stop=True)
                ot = osb.tile([128, 1024], f32, name="ot", tag="ot")
                if tg % 2 == 0:
                    nc.scalar.activation(ot[:], tp[:], AF.Copy)
                else:
                    nc.vector.tensor_copy(ot[:], tp[:])
                nc.sync.dma_start(
                    out=io["out"][b, tg * 512:(tg + 1) * 512, :].rearrange(
                        "(q p) d -> p q d", p=128),
                    in_=ot[:].rearrange("p (q d) -> p q d", d=D))

        units = []
        for p in range(1, MAX_SKIP + 1):
            nw = T // p
            for ws in range(0, nw, 512):
                for t in range(p):
                    for j in range(G):
                        units.append((p, ws, t, j))

        # NOTE: PRE+DEFER must stay <= 3 units; at PRE=2,DEFER=2 the cross-pass
        # gather prefetch overtakes the deferred writeback tail (stale reads).
        DEFER = 1
        PRE = 1
        pend_tail = []
        xcs = {n: emit_gather(*units[n]) for n in range(PRE)}
        for n, u in enumerate(units):
            if n + PRE < len(units):
                xcs[n + PRE] = emit_gather(*units[n + PRE])
            tail = emit_head(*u, xcs.pop(n))
            pend_tail.append(tail)
            if len(pend_tail) > DEFER:
                done = pend_tail.pop(0)
                emit_tail(*done)
                if done[0] == MAX_SKIP and done[2] == MAX_SKIP - 1:
                    emit_transpose(done[3])
        while pend_tail:
            done = pend_tail.pop(0)
            emit_tail(*done)
            if done[0] == MAX_SKIP and done[2] == MAX_SKIP - 1:
                emit_transpose(done[3])


def _prep_host(inputs):
    import ml_dtypes
    bf16 = ml_dtypes.bfloat16
    f8 = ml_dtypes.float8_e4m3fn

    def q8(a, scale):
        return np.ascontiguousarray(
            np.clip(np.asarray(a, dtype=np.float32) * scale, -240, 240)).astype(f8)

    x = np.asarray(inputs["x"], dtype=np.float32)
    adj = np.asarray(inputs["adj"], dtype=np.float32)
    adjT = q8(adj.T, AS)
    rs = np.ascontiguousarray(
        adj.sum(axis=1, dtype=np.float32).reshape(1, T)).astype(bf16)
    wg0 = np.ascontiguousarray(inputs["Wg0"], dtype=np.float32).astype(bf16)
    wg1 = np.ascontiguousarray(inputs["Wg1"], dtype=np.float32).astype(bf16)
    bg0 = np.ascontiguousarray(inputs["bg0"], dtype=np.float32).reshape(1, H).astype(bf16)
    bg1 = np.ascontiguousarray(inputs["bg1"], dtype=np.float32).reshape(1, H).astype(bf16)
    wihT = np.ascontiguousarray(
        np.asarray(inputs["Wih"], dtype=np.float32).transpose(0, 2, 1)).astype(bf16)
    whhT = np.ascontiguousarray(
        np.asarray(inputs["Whh"], dtype=np.float32).transpose(0, 2, 1)).astype(bf16)
    bias = np.asarray(inputs["bih"], dtype=np.float32) + np.asarray(inputs["bhh"], dtype=np.float32)
    biasT = np.ascontiguousarray(
        bias.reshape(MAX_SKIP, 8, 128).transpose(2, 0, 1).reshape(128, MAX_SKIP * 8))
    shared = dict(adjT=adjT, rs=rs, wg0=wg0, wg1=wg1, bg0=bg0, bg1=bg1,
                  wihT=wihT, whhT=whhT, biasT=biasT)
    xb = q8(x, XS)
    in_maps = []
    for c in range(NCORES):
        m = dict(shared)
        m["x"] = np.ascontiguousarray(xb[c * BL:(c + 1) * BL])
        in_maps.append(m)
    return in_maps


def get_compiled():
    global _COMPILED
    if _COMPILED is None:
        _COMPILED = _build_program()
    return _COMPILED


def kernel(**inputs) -> np.ndarray:
    from concourse.bass_utils import run_bass_kernel_spmd

    nc = get_compiled()
    in_maps = _prep_host(inputs)
    res = run_bass_kernel_spmd(nc, in_maps, list(range(NCORES)))
    out = np.concatenate([res.results[c]["out"] for c in range(NCORES)], axis=0)
    return out.astype(np.float32)


# revision 35
# speedup vs baseline: 1.0856x; 1.0029x over previous
"""Trainium2 Bass kernel for the DTGL GCN+windowed-LSTM module (bf16 rewrite).

Computation (see reference):
  h = relu(adj @ (x @ Wg0 + bg0));  h = relu(adj @ (h @ Wg1 + bg1))
  for p in 1..4: run LSTM_p over disjoint length-p windows of h (zero init
  state), writing the last hidden state back at each window end (in place).

Sharding: pure data-parallel over batch B=64 across 8 cores (8 batches per
core); adj and all weights replicated. No collectives.

Perf design vs the fp32r baseline:
  - All matmul operands bf16 (PSUM accumulation stays f32): halves the
    moving-stream bytes, halves LDWEIGHTS time, halves SBUF/DMA footprint,
    and lowers PE power so the HAM clock-gate stays at full rate.
  - adjT is SBUF-resident (loaded once, bf16, 64KB/partition).
  - Software-pipelined phases: 1B(j-1) matmuls fill the PE pipe while
    1A(j)'s PSUM->SBUF copies drain (same for 2A/2B), so the PE never
    waits on a copy.
  - LSTM runs batches round-robin per timestep so PE matmuls of batch j+1
    overlap ACT/DVE/Pool cell math of batch j. Gate PSUM uses all 8 banks.
  - Output transpose via identity matmul (stationary = h2T block, moving =
    identity) producing f32 PSUM directly.
"""

import numpy as np

B, T, D, H = 64, 2048, 256, 256
MAX_SKIP = 4
NCORES = 8
BL = B // NCORES          # batches per core
G = 4                     # batches per group
NGRP = BL // G
TK = T // 128             # 16 t-chunks
JB0 = {"i": 0, "f": 2, "g": 4, "o": 6}
XS = 16.0                 # fp8 quantization scales: x, adjT, h1
AS = 8192.0
HS = 64.0

_COMPILED = None


def _build_program():
    import concourse.mybir as mybir
    import concourse.tile as tile
    from concourse import bacc

    f32 = mybir.dt.float32
    bf16 = mybir.dt.bfloat16
    f8 = mybir.dt.float8e4

    nc = bacc.Bacc("TRN2", target_bir_lowering=False, debug=False)

    io = dict(
        x=nc.dram_tensor("x", [BL, T, D], f8, kind="ExternalInput").ap(),
        adjT=nc.dram_tensor("adjT", [T, T], f8, kind="ExternalInput").ap(),
        rs=nc.dram_tensor("rs", [1, T], bf16, kind="ExternalInput").ap(),
        wg0=nc.dram_tensor("wg0", [D, H], bf16, kind="ExternalInput").ap(),
        wg1=nc.dram_tensor("wg1", [D, H], bf16, kind="ExternalInput").ap(),
        bg0=nc.dram_tensor("bg0", [1, H], bf16, kind="ExternalInput").ap(),
        bg1=nc.dram_tensor("bg1", [1, H], bf16, kind="ExternalInput").ap(),
        wihT=nc.dram_tensor("wihT", [MAX_SKIP, H, 4 * H], bf16, kind="ExternalInput").ap(),
        whhT=nc.dram_tensor("whhT", [MAX_SKIP, H, 4 * H], bf16, kind="ExternalInput").ap(),
        biasT=nc.dram_tensor("biasT", [128, MAX_SKIP * 8], f32, kind="ExternalInput").ap(),
        out=nc.dram_tensor("out", [BL, T, D], f32, kind="ExternalOutput").ap(),
    )

    with tile.TileContext(nc) as tc:
        _emit(nc, tc, mybir, io)

    nc.compile()
    return nc


def _emit(nc, tc, mybir, io):
    from contextlib import ExitStack
    from concourse.masks import make_identity

    f32 = mybir.dt.float32
    bf16 = mybir.dt.bfloat16
    f8 = mybir.dt.float8e4
    AF = mybir.ActivationFunctionType

    with ExitStack() as root:
        cp = root.enter_context(tc.tile_pool(name="const", bufs=1))
        # adjT resident: 8 pair-tiles [128, 2*2048] fp8 (ksub-plane-major) for
        # DoubleRow matmuls: plane ks covers rows (2m+ks)*128..(2m+ks+1)*128.
        adjt = []
        for m in range(TK // 2):
            a = cp.tile([128, 2 * T], f8, name=f"adjt_{m}")
            for ks in range(2):
                nc.sync.dma_start(
                    out=a[:, ks * T:(ks + 1) * T],
                    in_=io["adjT"][(2 * m + ks) * 128:(2 * m + ks + 1) * 128, :])
            adjt.append(a)
        wg0_sb = cp.tile([128, 2 * H], bf16, name="wg0_sb")
        wg1_sb = cp.tile([128, 2 * H], bf16, name="wg1_sb")
        for dk in range(2):
            nc.sync.dma_start(out=wg0_sb[:, dk * H:(dk + 1) * H],
                              in_=io["wg0"][dk * 128:(dk + 1) * 128, :])
            nc.sync.dma_start(out=wg1_sb[:, dk * H:(dk + 1) * H],
                              in_=io["wg1"][dk * 128:(dk + 1) * 128, :])
        bg0_sb = cp.tile([1, H], bf16, name="bg0_sb")
        bg1_sb = cp.tile([1, H], bf16, name="bg1_sb")
        rs_sb = cp.tile([1, T], bf16, name="rs_sb")
        biasT_sb = cp.tile([128, MAX_SKIP * 8], f32, name="biasT_sb")
        nc.sync.dma_start(out=bg0_sb[:], in_=io["bg0"][:])
        nc.sync.dma_start(out=bg1_sb[:], in_=io["bg1"][:])
        nc.sync.dma_start(out=rs_sb[:], in_=io["rs"][:])
        nc.sync.dma_start(out=biasT_sb[:], in_=io["biasT"][:])
        id32 = cp.tile([128, 128], f32, name="id32")
        ident = cp.tile([128, 128], bf16, name="ident")
        make_identity(nc, id32[:])
        nc.vector.tensor_copy(ident[:], id32[:])

        h2t_pool = root.enter_context(tc.tile_pool(name="h2tp", bufs=1))
        lw_pool = root.enter_context(tc.tile_pool(name="lw", bufs=2))

        # all x tiles upfront: [128(t within k-block), k*D + d] fp8, one per batch
        xs_all = []
        for b in range(BL):
            xt = h2t_pool.tile([128, TK * D], f8, name=f"x_{b}", tag=f"x_{b}")
            nc.sync.dma_start(
                out=xt[:].rearrange("p (k d) -> p k d", d=D),
                in_=io["x"][b].rearrange("(k p) d -> p k d", p=128))
            xs_all.append(xt)

        for grp in range(NGRP):
            _group(nc, tc, io, f32, bf16, AF, grp, adjt, wg0_sb, wg1_sb,
                   bg0_sb, bg1_sb, rs_sb, biasT_sb, ident, h2t_pool, lw_pool,
                   xs_all)


def _group(nc, tc, io, f32, bf16, AF, grp, adjt, wg0_sb, wg1_sb, bg0_sb,
           bg1_sb, rs_sb, biasT_sb, ident, h2t_pool, lw_pool, xs_all):
    from contextlib import ExitStack
    import concourse.mybir as mybir

    f8 = mybir.dt.float8e4
    DR = mybir.MatmulPerfMode.DoubleRow
    bs = grp * G
    # h2T slabs: feature-major [h(part within hk), hk*T + t], bf16.
    h2t = [h2t_pool.tile([128, 2 * T], bf16, name=f"h2t_{j}", tag=f"h2t_{j}")
           for j in range(G)]

    with ExitStack() as gcn:
        h1_pool = gcn.enter_context(tc.tile_pool(name="h1p", bufs=1))
        # h1 slabs: row-major [u(part within ub), ub*H + h], fp8 scaled by HS.
        h1 = [h1_pool.tile([128, TK * H], f8, name=f"h1_{j}", tag=f"h1_{j}")
              for j in range(G)]
        z1_pool = gcn.enter_context(tc.tile_pool(name="z1p", bufs=2))

        # ---------------- Phase 1: layer 1 (1A + 1B pipelined) ----------------
        with ExitStack() as ph:
            zps = ph.enter_context(tc.tile_pool(name="zps", bufs=1, space="PSUM"))
            hps = ph.enter_context(tc.tile_pool(name="hps", bufs=4, space="PSUM"))

            xs = [xs_all[bs + j] for j in range(G)]

            z1t = {}   # (j, uh, dk) -> sbuf tile [128, 1024] bf16
            pend = []  # queue of emitted-1A halves awaiting 1B: (j, uh)

            def emit_1a(j, uh):
                zp = {(dk, q): zps.tile([128, 512], f32, name="zp", tag=f"zp{dk}{q}")
                      for dk in range(2) for q in range(2)}
                xv = xs[j][:].rearrange("p (k d) -> p k d", d=D)
                for m in range(TK // 2):
                    av = adjt[m][:].rearrange("p (k u) -> p k u", k=2)
                    for dk in range(2):
                        lhs = xv[:, 2 * m:2 * m + 2, dk * 128:(dk + 1) * 128]
                        for q in range(2):
                            us = uh * 1024 + q * 512
                            nc.tensor.matmul(
                                zp[(dk, q)][:], lhs, av[:, :, us:us + 512],
                                start=(m == 0), stop=(m == TK // 2 - 1),
                                perf_mode=DR)
                for dk in range(2):
                    zt = z1_pool.tile([128, 1024], bf16, name="z1t", tag=f"z1t{dk}")
                    z1t[(j, uh, dk)] = zt
                    nc.vector.tensor_scalar_mul(zt[:, 0:512], zp[(dk, 0)][:], 1.0 / (XS * AS))
                    nc.vector.tensor_scalar_mul(zt[:, 512:1024], zp[(dk, 1)][:], 1.0 / (XS * AS))

            def emit_1b(j, uh):
                for ub_l in range(8):
                    ub = uh * 8 + ub_l
                    hp = hps.tile([128, H], f32, name="hp", tag="hp")
                    for dk in range(2):
                        nc.tensor.matmul(
                            hp[:], z1t[(j, uh, dk)][:, ub_l * 128:(ub_l + 1) * 128],
                            wg0_sb[:, dk * H:(dk + 1) * H],
                            start=(dk == 0), stop=False)
                    nc.tensor.matmul(hp[:], rs_sb[0:1, ub * 128:(ub + 1) * 128],
                                     bg0_sb[0:1, :], start=False, stop=True)
                    # h1 = relu(HS * z) in fp8 (scale folded through relu)
                    nc.scalar.activation(h1[j][:, ub * H:(ub + 1) * H], hp[:],
                                         AF.Relu, scale=HS)

            for j in range(G):
                for uh in range(2):
                    emit_1a(j, uh)
                    pend.append((j, uh))
                    if len(pend) > 1:
                        emit_1b(*pend.pop(0))
            while pend:
                emit_1b(*pend.pop(0))

        # ---------------- Phase 2: layer 2 (2A + 2B pipelined) ----------------
        with ExitStack() as ph:
            zps = ph.enter_context(tc.tile_pool(name="zps2", bufs=1, space="PSUM"))
            hps = ph.enter_context(tc.tile_pool(name="hps2", bufs=1, space="PSUM"))
            z2_pool = ph.enter_context(tc.tile_pool(name="z2p", bufs=2))

            z2t = {}
            pend = []

            def emit_2a(j, uh):
                zp = {(hk, q): zps.tile([128, 512], f32, name="zp2", tag=f"zp2{hk}{q}")
                      for hk in range(2) for q in range(2)}
                hv = h1[j][:].rearrange("p (u h) -> p u h", h=H)
                for m in range(TK // 2):
                    av = adjt[m][:].rearrange("p (k u) -> p k u", k=2)
                    for hk in range(2):
                        lhs = hv[:, 2 * m:2 * m + 2, hk * 128:(hk + 1) * 128]
                        for q in range(2):
                            us = uh * 1024 + q * 512
                            nc.tensor.matmul(
                                zp[(hk, q)][:], lhs, av[:, :, us:us + 512],
                                start=(m == 0), stop=(m == TK // 2 - 1),
                                perf_mode=DR)
                for hk in range(2):
                    zt = z2_pool.tile([128, 1024], bf16, name="z2t", tag=f"z2t{hk}")
                    z2t[(j, uh, hk)] = zt
                    nc.vector.tensor_scalar_mul(zt[:, 0:512], zp[(hk, 0)][:], 1.0 / (HS * AS))
                    nc.vector.tensor_scalar_mul(zt[:, 512:1024], zp[(hk, 1)][:], 1.0 / (HS * AS))

            def emit_2b(j, uh):
                for ho in range(2):
                    for q in range(2):
                        hp = hps.tile([128, 512], f32, name="hp2", tag=f"hp2{ho}{q}")
                        for hk in range(2):
                            nc.tensor.matmul(
                                hp[:], wg1_sb[:, hk * H + ho * 128: hk * H + (ho + 1) * 128],
                                z2t[(j, uh, hk)][:, q * 512:(q + 1) * 512],
                                start=(hk == 0), stop=False)
                        us = uh * 1024 + q * 512
                        nc.tensor.matmul(hp[:], bg1_sb[0:1, ho * 128:(ho + 1) * 128],
                                         rs_sb[0:1, us:us + 512], start=False, stop=True)
                        nc.vector.tensor_relu(h2t[j][:, ho * T + us: ho * T + us + 512],
                                              hp[:])

            for j in range(G):
                for uh in range(2):
                    emit_2a(j, uh)
                    pend.append((j, uh))
                    if len(pend) > 1:
                        emit_2b(*pend.pop(0))
            while pend:
                emit_2b(*pend.pop(0))

    # ---------------- Phases 3-4: the four LSTM passes ----------------
    # Two cohorts (j0,j1 | j2,j3) run with a one-pass skew so ACT-heavy t=0
    # units (esp. all of p=1) interleave with PE-heavy t>0 units.
    with ExitStack() as ph:
        gps = ph.enter_context(tc.tile_pool(name="gps", bufs=1, space="PSUM"))
        gsb = ph.enter_context(tc.tile_pool(name="gsb", bufs=3))
        st_pool = ph.enter_context(tc.tile_pool(name="st", bufs=1))
        h_pool = ph.enter_context(tc.tile_pool(name="hs", bufs=2))
        gx_pool = ph.enter_context(tc.tile_pool(name="gx", bufs=3))

        c_st = [st_pool.tile([128, 1024], bf16, name=f"c_{j}", tag=f"c{j}")
                for j in range(G)]
        h_t = [None] * G
        lw = {}

        def get_weights(p):
            if p in lw:
                return lw[p]
            wih = lw_pool.tile([128, 2 * 4 * H], bf16, name=f"wih{grp}{p}", tag="wih")
            for hk in range(2):
                nc.sync.dma_start(out=wih[:, hk * 4 * H:(hk + 1) * 4 * H],
                                  in_=io["wihT"][p - 1, hk * 128:(hk + 1) * 128, :])
            whh = None
            if p > 1:
                whh = lw_pool.tile([128, 2 * 4 * H], bf16, name=f"whh{grp}{p}", tag="whh")
                for hk in range(2):
                    nc.sync.dma_start(out=whh[:, hk * 4 * H:(hk + 1) * 4 * H],
                                      in_=io["whhT"][p - 1, hk * 128:(hk + 1) * 128, :])
            lw[p] = (wih, whh)
            return lw[p]

        def _mkview(p, j):
            nw = T // p
            return [h2t[j][:, hk * T: hk * T + nw * p].rearrange(
                "a (w q) -> a w q", q=p) for hk in range(2)]

        def emit_gather(p, ws, t, j):
            if p == 1:
                return None
            nw = T // p
            ncw = min(512, nw - ws)
            view = _mkview(p, j)
            xc = gx_pool.tile([128, 1024], bf16, name="xc", tag="xc")
            nc.gpsimd.tensor_copy(xc[:, 0:ncw],
                                  view[0][:, ws:ws + ncw, t:t + 1])
            nc.vector.tensor_copy(xc[:, 512:512 + ncw],
                                  view[1][:, ws:ws + ncw, t:t + 1])
            return xc

        def emit_head(p, ws, t, j, xc):
            nw = T // p
            ncw = min(512, nw - ws)
            spans = ([slice(0, 1024)] if ncw == 512
                     else [slice(0, ncw), slice(512, 512 + ncw)])
            wih, whh = get_weights(p)
            view = _mkview(p, j)
            if True:
                    if True:
                        gates = "igo" if t == 0 else "ifgo"
                        gp = {}
                        # input-weight matmuls first (no state dependency)
                        for gn in gates:
                            psum = gps.tile([128, 1024], f32, name=f"ps_{gn}", tag=f"ps_{gn}")
                            gp[gn] = psum
                            for half in range(2):
                                jb = JB0[gn] + half
                                o = psum[:, half * 512: half * 512 + ncw]
                                for hk in range(2):
                                    rhs = (view[hk][:, ws:ws + ncw, 0:1] if p == 1
                                           else xc[:, hk * 512: hk * 512 + ncw])
                                    nc.tensor.matmul(
                                        o,
                                        wih[:, hk * 4 * H + jb * 128: hk * 4 * H + (jb + 1) * 128],
                                        rhs,
                                        start=(hk == 0),
                                        stop=(t == 0 and hk == 1))
                        if t > 0:
                            for gn in gates:
                                for half in range(2):
                                    jb = JB0[gn] + half
                                    o = gp[gn][:, half * 512: half * 512 + ncw]
                                    for hk in range(2):
                                        nc.tensor.matmul(
                                            o,
                                            whh[:, hk * 4 * H + jb * 128: hk * 4 * H + (jb + 1) * 128],
                                            h_t[j][:, hk * 512: hk * 512 + ncw],
                                            start=False, stop=(hk == 1))
                        act = {}
                        for gn in gates:
                            fn = AF.Tanh if gn == "g" else AF.Sigmoid
                            a = gsb.tile([128, 1024], bf16, name=f"a_{gn}", tag=f"a_{gn}")
                            act[gn] = a
                            for half in range(2):
                                col = (p - 1) * 8 + JB0[gn] + half
                                nc.scalar.activation(
                                    a[:, half * 512: half * 512 + ncw],
                                    gp[gn][:, half * 512: half * 512 + ncw],
                                    fn, bias=biasT_sb[:, col:col + 1])
                        cn = c_st[j]
                        if t == 0:
                            for s in spans:
                                nc.vector.tensor_mul(cn[:, s], act["i"][:, s], act["g"][:, s])
                        else:
                            for s in spans:
                                nc.vector.tensor_mul(act["g"][:, s], act["i"][:, s], act["g"][:, s])
                            for s in spans:
                                nc.gpsimd.tensor_mul(cn[:, s], act["f"][:, s], cn[:, s])
                            for s in spans:
                                nc.vector.tensor_add(cn[:, s], cn[:, s], act["g"][:, s])
                        # tail (tanh + h-mul) deferred one unit to avoid ACT
                        # head-of-line blocking on the not-yet-ready c input
                        return (p, ws, t, j, ncw, spans, view, act, cn)

        def emit_tail(p, ws, t, j, ncw, spans, view, act, cn):
            # tanh(c) overwrites the i tile (free after c update)
            tc_t = act["i"]
            for s in spans:
                nc.scalar.activation(tc_t[:, s], cn[:, s], AF.Tanh)
            if t == p - 1:
                for hk in range(2):
                    nc.vector.tensor_mul(
                        view[hk][:, ws:ws + ncw, p - 1:p],
                        act["o"][:, hk * 512: hk * 512 + ncw],
                        tc_t[:, hk * 512: hk * 512 + ncw])
            else:
                hn = h_pool.tile([128, 1024], bf16, name="hn", tag=f"h{j}")
                for s in spans:
                    nc.vector.tensor_mul(hn[:, s], act["o"][:, s], tc_t[:, s])
                h_t[j] = hn

        # ---- transpose-out emitter (phase 5), interleaved into the p=4 tail ----
        osb = ph.enter_context(tc.tile_pool(name="osb", bufs=2))
        tptags = ["ps_i", "ps_f", "ps_g", "ps_o"]

        def emit_transpose(j):
            b = bs + j
            for tg in range(4):
                tp = gps.tile([128, 1024], f32, name="tp", tag=tptags[tg])
                for q in range(4):
                    tk = tg * 4 + q
                    for hk in range(2):
                        nc.tensor.matmul(
                            tp[:, q * D + hk * 128: q * D + (hk + 1) * 128],
                            h2t[j][:, hk * T + tk * 128: hk * T + (tk + 1) * 128],
                            ident[:], start=True, stop=True)
                ot = osb.tile([128, 1024], f32, name="ot", tag="ot")
                if tg % 2 == 0:
                    nc.scalar.activation(ot[:], tp[:], AF.Copy)
                else:
                    nc.vector.tensor_copy(ot[:], tp[:])
                nc.sync.dma_start(
                    out=io["out"][b, tg * 512:(tg + 1) * 512, :].rearrange(
                        "(q p) d -> p q d", p=128),
                    in_=ot[:].rearrange("p (q d) -> p q d", d=D))

        units = []
        for p in range(1, MAX_SKIP + 1):
            nw = T // p
            for ws in range(0, nw, 512):
                for t in range(p):
                    for j in range(G):
                        units.append((p, ws, t, j))

        # NOTE: PRE+DEFER must stay <= 3 units; at PRE=2,DEFER=2 the cross-pass
        # gather prefetch overtakes the deferred writeback tail (stale reads).
        DEFER = 1
        PRE = 2
        pend_tail = []
        xcs = {n: emit_gather(*units[n]) for n in range(PRE)}
        for n, u in enumerate(units):
            if n + PRE < len(units):
                xcs[n + PRE] = emit_gather(*units[n + PRE])
            tail = emit_head(*u, xcs.pop(n))
            pend_tail.append(tail)
            if len(pend_tail) > DEFER:
                done = pend_tail.pop(0)
                emit_tail(*done)
                if done[0] == MAX_SKIP and done[2] == MAX_SKIP - 1:
                    emit_transpose(done[3])
        while pend_tail:
            done = pend_tail.pop(0)
            emit_tail(*done)
            if done[0] == MAX_SKIP and done[2] == MAX_SKIP - 1:
                emit_transpose(done[3])


def _prep_host(inputs):
    import ml_dtypes
    bf16 = ml_dtypes.bfloat16
    f8 = ml_dtypes.float8_e4m3fn

    def q8(a, scale):
        return np.ascontiguousarray(
            np.clip(np.asarray(a, dtype=np.float32) * scale, -240, 240)).astype(f8)

    x = np.asarray(inputs["x"], dtype=np.float32)
    adj = np.asarray(inputs["adj"], dtype=np.float32)
    adjT = q8(adj.T, AS)
    rs = np.ascontiguousarray(
        adj.sum(axis=1, dtype=np.float32).reshape(1, T)).astype(bf16)
    wg0 = np.ascontiguousarray(inputs["Wg0"], dtype=np.float32).astype(bf16)
    wg1 = np.ascontiguousarray(inputs["Wg1"], dtype=np.float32).astype(bf16)
    bg0 = np.ascontiguousarray(inputs["bg0"], dtype=np.float32).reshape(1, H).astype(bf16)
    bg1 = np.ascontiguousarray(inputs["bg1"], dtype=np.float32).reshape(1, H).astype(bf16)
    wihT = np.ascontiguousarray(
        np.asarray(inputs["Wih"], dtype=np.float32).transpose(0, 2, 1)).astype(bf16)
    whhT = np.ascontiguousarray(
        np.asarray(inputs["Whh"], dtype=np.float32).transpose(0, 2, 1)).astype(bf16)
    bias = np.asarray(inputs["bih"], dtype=np.float32) + np.asarray(inputs["bhh"], dtype=np.float32)
    biasT = np.ascontiguousarray(
        bias.reshape(MAX_SKIP, 8, 128).transpose(2, 0, 1).reshape(128, MAX_SKIP * 8))
    shared = dict(adjT=adjT, rs=rs, wg0=wg0, wg1=wg1, bg0=bg0, bg1=bg1,
                  wihT=wihT, whhT=whhT, biasT=biasT)
    xb = q8(x, XS)
    in_maps = []
    for c in range(NCORES):
        m = dict(shared)
        m["x"] = np.ascontiguousarray(xb[c * BL:(c + 1) * BL])
        in_maps.append(m)
    return in_maps


def get_compiled():
    global _COMPILED
    if _COMPILED is None:
        _COMPILED = _build_program()
    return _COMPILED


def kernel(**inputs) -> np.ndarray:
    from concourse.bass_utils import run_bass_kernel_spmd

    nc = get_compiled()
    in_maps = _prep_host(inputs)
    res = run_bass_kernel_spmd(nc, in_maps, list(range(NCORES)))
    out = np.concatenate([res.results[c]["out"] for c in range(NCORES)], axis=0)
    return out.astype(np.float32)
